# revision 7
# baseline (speedup 1.0000x reference)
"""Trainium2 Bass kernel for bucketed causal linear self-attention.

Model (B=4, T=4096, DIM=1024, H=16 heads, E=64, BUCKET=64):
  q,k,v = x@Wq, x@Wk, x@Wv ; q softmaxed over head-dim, k -> elu(k)+1
  per-bucket context C_u = cumsum_u(k_bu^T v_bu), normalized by cumsum of
  key-sums, shifted one bucket; attn_bu = q_bu @ C_{u-1}; out = attn@Wo + bo.

Sharding over 8 cores: core c -> batch c//2, head-group c%2 (8 heads = 512
feats). q/k/v column-sharded by head, Wo row-sharded; host sums the two
partial outputs per batch (all-reduce on host) and adds bo.

v3 structure per core:
  x arrives HOST-TRANSPOSED [DIM, T] so x^T tiles load as plain DMAs (the
  v2 dma_start_transpose chain serialized on the xbar and re-throttled the
  PE clock).  attn^T accumulates on the tensor engine:
      C_carry^T @ q2  +  sum_j S_j^T @ q2[:, buckets > j]
  (shrinking-N matmuls; no per-bucket DVE walk).  The chunk context
  increment is a DVE tensor_reduce over the bf16 S tile; the bucket-ksum
  prefix scan runs on the same tile.  Softmax reciprocal is one DVE
  reciprocal per chunk; its per-token broadcast is a 2-row matmul.
  q'' = exp(q) * softmax_recip * 1/(ksum_prefix+eps), bucket-0 blindspot
  via a zeroed scale column on chunk 0.  Output is bf16; host sums the
  two per-batch partials in f32 and adds bo.
"""

import sys
import numpy as np
import ml_dtypes

sys.path.insert(0, "/opt/trn_rl_repo")

B, T, DIM, H, BUCKET = 4, 4096, 1024, 16, 64
E = 64           # head dim
HC = 8           # heads per core
F = HC * E       # per-core feature width = 512
CH = 512         # tokens per chunk
UC = CH // BUCKET  # buckets per chunk = 8
PAIRS = HC // 2  # head pairs = 4
KT = DIM // 128  # contraction tiles = 8
EPS = 1e-6

_NC_CACHE = {}


def build_nc(n_chunks=T // CH):
    import concourse.bass as bass
    import concourse.mybir as mybir
    from concourse import bacc
    from concourse.tile import TileContext

    BF16 = mybir.dt.bfloat16
    F32 = mybir.dt.float32
    AF = mybir.ActivationFunctionType
    OP = mybir.AluOpType

    Tt = n_chunks * CH

    nc = bacc.Bacc("TRN2", target_bir_lowering=False, debug=False, num_devices=8)
    x = nc.dram_tensor("x", [DIM, Tt], BF16, kind="ExternalInput").ap()
    wq = nc.dram_tensor("wq", [DIM, F], BF16, kind="ExternalInput").ap()
    wk = nc.dram_tensor("wk", [DIM, F], BF16, kind="ExternalInput").ap()
    wv = nc.dram_tensor("wv", [DIM, F], BF16, kind="ExternalInput").ap()
    wo = nc.dram_tensor("wo", [F, DIM], BF16, kind="ExternalInput").ap()
    orp = nc.dram_tensor("orp", [128, 128], BF16, kind="ExternalInput").ap()
    out = nc.dram_tensor("out", [Tt, DIM], BF16, kind="ExternalOutput").ap()

    with TileContext(nc) as tc:
        with tc.tile_pool(name="const", bufs=1) as constp, \
             tc.tile_pool(name="xt", bufs=2) as xtp, \
             tc.tile_pool(name="act", bufs=2) as actp, \
             tc.tile_pool(name="tmp", bufs=3) as tmpp, \
             tc.tile_pool(name="small", bufs=8) as smallp, \
             tc.tile_pool(name="outp", bufs=2) as outp, \
             tc.tile_pool(name="ps_proj", bufs=2, space="PSUM") as psproj, \
             tc.tile_pool(name="ps_s", bufs=1, space="PSUM") as pss, \
             tc.tile_pool(name="ps_attn", bufs=2, space="PSUM") as psattn, \
             tc.tile_pool(name="ps_misc", bufs=2, space="PSUM") as psmisc:

            # ---- resident constants ----
            wq_sb = constp.tile([128, KT, F], BF16, tag="wq")
            wk_sb = constp.tile([128, KT, F], BF16, tag="wk")
            wv_sb = constp.tile([128, KT, F], BF16, tag="wv")
            wo_sb = constp.tile([128, PAIRS, DIM], BF16, tag="wo")
            nc.gpsimd.dma_start(out=wq_sb[:], in_=wq.rearrange("(kt p) f -> p kt f", p=128))
            nc.gpsimd.dma_start(out=wk_sb[:], in_=wk.rearrange("(kt p) f -> p kt f", p=128))
            nc.gpsimd.dma_start(out=wv_sb[:], in_=wv.rearrange("(kt p) f -> p kt f", p=128))
            nc.gpsimd.dma_start(out=wo_sb[:], in_=wo.rearrange("(ft p) n -> p ft n", p=128))

            ones_sum = constp.tile([128, 2], BF16, tag="ones_sum")
            nc.vector.memset(ones_sum[:], 0.0)
            nc.vector.memset(ones_sum[0:64, 0:1], 1.0)
            nc.vector.memset(ones_sum[64:128, 1:2], 1.0)
            # orp[32p+0, 0:64]=1, orp[32p+1, 64:128]=1 (host-built)
            orp_sb = constp.tile([128, 128], BF16, tag="orp")
            nc.gpsimd.dma_start(out=orp_sb[:], in_=orp[:])

            # running context (+ ksum col 64) per pair, f32 master + bf16 copy
            c_ms = constp.tile([128, PAIRS, E + 1], F32, tag="c_ms")
            nc.vector.memset(c_ms[:], 0.0)
            c_bf = constp.tile([128, PAIRS, E], BF16, tag="c_bf")
            nc.vector.memset(c_bf[:], 0.0)

            state = {}

            def emit_proj(c):
                xT = xtp.tile([128, KT, CH], BF16, tag="xT")
                nc.sync.dma_start(
                    out=xT[:],
                    in_=x[:, c * CH:(c + 1) * CH].rearrange(
                        "(kt p) t -> p kt t", p=128))

                # q^T, exp(q), per-token softmax sums (rows 32p..32p+2 of sm)
                E_sb = actp.tile([128, PAIRS, CH], BF16, tag="E")
                sm = psmisc.tile([128, CH], F32, tag="misc")
                for p in range(PAIRS):
                    qt = psproj.tile([128, CH], F32, tag="proj")
                    for kt in range(KT):
                        nc.tensor.matmul(qt[:], wq_sb[:, kt, p * 128:(p + 1) * 128],
                                         xT[:, kt, :], start=(kt == 0), stop=(kt == KT - 1))
                    nc.scalar.activation(out=E_sb[:, p, :], in_=qt[:], func=AF.Exp)
                    nc.tensor.matmul(sm[32 * p:32 * p + 2, :], ones_sum[:], E_sb[:, p, :],
                                     start=True, stop=True, tile_position=(0, 32 * p))
                recip_sb = actp.tile([128, CH], BF16, tag="recip")
                with nc.allow_low_precision(reason="bf16 softmax recip, 4e-3 rel"):
                    nc.vector.reciprocal(out=recip_sb[0:98, :], in_=sm[0:98, :])

                psik = actp.tile([128, PAIRS, CH], BF16, tag="psik")
                v_sb = actp.tile([128, PAIRS, HC * (E + 1) // PAIRS * PAIRS], BF16, tag="v")
                # v_sb free layout per tok-tile: 8 heads x 65 (64 v + ones col)
                for tt in range(PAIRS):  # 4 token tiles of 128
                    kp = psproj.tile([128, F], F32, tag="proj")
                    for kt in range(KT):
                        nc.tensor.matmul(kp[:], xT[:, kt, tt * 128:(tt + 1) * 128],
                                         wk_sb[:, kt, :], start=(kt == 0), stop=(kt == KT - 1))
                    tm = tmpp.tile([128, F], F32, tag="tm")
                    nc.vector.tensor_scalar_min(tm[:], kp[:], 0.0)
                    tm2 = tmpp.tile([128, F], F32, tag="tm2")
                    nc.scalar.activation(out=tm2[:], in_=tm[:], func=AF.Exp)
                    # psi = max(k,0) + exp(min(k,0))
                    nc.vector.scalar_tensor_tensor(
                        out=psik[:, tt, :], in0=kp[:], scalar=0.0, in1=tm2[:],
                        op0=OP.max, op1=OP.add)

                    vp = psproj.tile([128, F], F32, tag="proj")
                    for kt in range(KT):
                        nc.tensor.matmul(vp[:], xT[:, kt, tt * 128:(tt + 1) * 128],
                                         wv_sb[:, kt, :], start=(kt == 0), stop=(kt == KT - 1))
                    v3 = v_sb[:, tt, :].rearrange("p (h e1) -> p h e1", e1=E + 1)
                    nc.scalar.activation(
                        out=v3[:, :, 0:E],
                        in_=vp[:].rearrange("p (h e) -> p h e", e=E), func=AF.Copy)
                    nc.vector.memset(v3[:, :, E:E + 1], 1.0)
                state[c] = (E_sb, recip_sb, psik, v_sb)

            def emit_attn(c):
                E_sb, recip_sb, psik, v_sb = state.pop(c)
                atn = actp.tile([128, PAIRS, CH], BF16, tag="atn")
                import concourse.bass as bass_mod

                # phase A: per-bucket context matmuls + scan chains, all pairs
                sbfs, kscs, q2s = [], [], []
                for p in range(PAIRS):
                    # S_j = psi_bu^T @ [v_bu | 1]
                    s_ev = pss.tile([128, UC // 2, E + 1], F32, tag="s_ev")
                    s_od = pss.tile([128, UC // 2, E + 1], F32, tag="s_od")
                    for j in range(UC):
                        sdst = s_ev if j % 2 == 0 else s_od
                        tt, r0 = j // 2, (j % 2) * 64
                        for hh in range(2):
                            h = 2 * p + hh
                            nc.tensor.matmul(
                                sdst[hh * 64:(hh + 1) * 64, j // 2, :],
                                psik[r0:r0 + 64, tt, h * E:(h + 1) * E],
                                v_sb[r0:r0 + 64, tt, :].rearrange(
                                    "p (g e1) -> p g e1", e1=E + 1)[:, h, :],
                                start=True, stop=True,
                                tile_position=(r0, hh * 64))

                    # all 8 buckets -> bf16 [128, (a 4) (b 2) 65]; b = j parity
                    s_bf = tmpp.tile([128, UC // 2, 2, E + 1], BF16, tag=f"s_bf{p}")
                    nc.vector.tensor_copy(out=s_bf[:, :, 0, :], in_=s_ev[:])
                    nc.vector.tensor_copy(out=s_bf[:, :, 1, :], in_=s_od[:])

                    # ksum exclusive prefix along buckets -> per-bucket scale R
                    ksv = s_bf[:].rearrange("p a b e1 -> p (a b) e1")[:, :, E:E + 1] \
                        .rearrange("p j one -> p (j one)")
                    ksc = smallp.tile([128, UC], F32, tag=f"ksc{p}")
                    nc.vector.tensor_tensor_scan(
                        out=ksc[:], data0=ksv, data1=ksv,
                        initial=c_ms[:, p, E:E + 1], op0=OP.add, op1=OP.bypass)
                    rs = smallp.tile([128, UC], F32, tag="rs")
                    nc.vector.tensor_copy(out=rs[:, 1:UC], in_=ksc[:, 0:UC - 1])
                    nc.vector.tensor_copy(out=rs[:, 0:1], in_=c_ms[:, p, E:E + 1])
                    R = smallp.tile([128, UC], BF16, tag="R")
                    nc.vector.tensor_scalar_add(rs[:], rs[:], EPS)
                    with nc.allow_low_precision(reason="bf16 ksum recip, 4e-3 rel"):
                        nc.vector.reciprocal(out=R[:], in_=rs[:])
                    if c == 0:
                        nc.vector.memset(R[:, 0:1], 0.0)  # bucket-0 blindspot

                    # q'' = exp(q) * softmax_recip * ksum_recip
                    repl = psmisc.tile([128, CH], F32, tag="misc")
                    nc.tensor.matmul(repl[:], orp_sb[32 * p:32 * p + 2, :],
                                     recip_sb[32 * p:32 * p + 2, :],
                                     start=True, stop=True, tile_position=(32 * p, 0))
                    rap = R[:]
                    Rb = bass_mod.AP(tensor=rap.tensor, offset=rap.offset,
                                     ap=[rap.ap[0], rap.ap[1], [0, BUCKET]])
                    RR = tmpp.tile([128, CH], BF16, tag="RR")
                    nc.vector.tensor_tensor(
                        out=RR[:].rearrange("p (u t) -> p u t", t=BUCKET),
                        in0=repl[:].rearrange("p (u t) -> p u t", t=BUCKET),
                        in1=Rb, op=OP.mult)
                    q2 = tmpp.tile([128, CH], BF16, tag=f"q2{p}")
                    nc.vector.tensor_tensor(out=q2[:], in0=E_sb[:, p, :], in1=RR[:],
                                            op=OP.mult)
                    sbfs.append(s_bf); kscs.append(ksc); q2s.append(q2)

                # phase B: attn^T accumulation per pair (q2 chains resolved by now)
                for p in range(PAIRS):
                    s_bf, ksc, q2 = sbfs[p], kscs[p], q2s[p]
                    at = psattn.tile([128, CH], F32, tag="attn")
                    for hh in range(2):
                        r0 = hh * 64
                        nc.tensor.matmul(
                            at[r0:r0 + 64, :], c_bf[r0:r0 + 64, p, :],
                            q2[r0:r0 + 64, :], start=True, stop=False,
                            tile_position=(r0, r0))
                        for j in range(UC - 1):
                            q0 = (j + 1) * BUCKET
                            nc.tensor.matmul(
                                at[r0:r0 + 64, q0:CH],
                                s_bf[r0:r0 + 64, j // 2, j % 2, 0:E],
                                q2[r0:r0 + 64, q0:CH], start=False,
                                stop=(j == UC - 2), tile_position=(r0, r0))
                    nc.vector.tensor_copy(out=atn[:, p, :], in_=at[:])

                    # advance the running carry (reads above see the old state)
                    c_red = smallp.tile([128, E], F32, tag="c_red")
                    nc.vector.tensor_reduce(
                        out=c_red[:],
                        in_=s_bf[:].rearrange("p a b e1 -> p e1 (a b)")[:, 0:E, :],
                        axis=mybir.AxisListType.X, op=OP.add)
                    nc.vector.tensor_tensor(out=c_ms[:, p, 0:E], in0=c_ms[:, p, 0:E],
                                            in1=c_red[:], op=OP.add)
                    nc.vector.tensor_copy(out=c_ms[:, p, E:E + 1], in_=ksc[:, UC - 1:UC])
                    nc.vector.tensor_copy(out=c_bf[:, p, :], in_=c_ms[:, p, 0:E])

                # output projection: out_chunk = attn^T.T @ Wo (contract feats)
                osb = outp.tile([128, PAIRS, DIM], BF16, tag="osb")
                for tt in range(PAIRS):
                    for half in range(2):
                        op_ = psproj.tile([128, 512], F32, tag="proj")
                        for p in range(PAIRS):
                            nc.tensor.matmul(
                                op_[:], atn[:, p, tt * 128:(tt + 1) * 128],
                                wo_sb[:, p, half * 512:(half + 1) * 512],
                                start=(p == 0), stop=(p == PAIRS - 1))
                        nc.scalar.activation(
                            out=osb[:, tt, half * 512:(half + 1) * 512],
                            in_=op_[:], func=AF.Copy)
                nc.gpsimd.dma_start(
                    out=out[c * CH:(c + 1) * CH, :].rearrange(
                        "(tt p) d -> p tt d", p=128),
                    in_=osb[:])

            for c in range(n_chunks + 1):
                if c < n_chunks:
                    emit_proj(c)
                if c >= 1:
                    emit_attn(c - 1)

    nc.finalize()
    return nc


def _orp():
    m = np.zeros((128, 128), dtype=ml_dtypes.bfloat16)
    for p in range(PAIRS):
        m[32 * p, 0:64] = 1
        m[32 * p + 1, 64:128] = 1
    return m


def build_in_maps(x, Wq, Wk, Wv, Wo):
    bf = ml_dtypes.bfloat16
    x = np.asarray(x)
    Wq, Wk, Wv, Wo = (np.asarray(w) for w in (Wq, Wk, Wv, Wo))
    in_maps = []
    for c in range(8):
        b, g = c // 2, c % 2
        sl = slice(g * F, (g + 1) * F)
        in_maps.append({
            "x": np.ascontiguousarray(x[b].T).astype(bf),
            "wq": Wq[:, sl].astype(bf),
            "wk": Wk[:, sl].astype(bf),
            "wv": Wv[:, sl].astype(bf),
            "wo": Wo[sl, :].astype(bf),
            "orp": _orp(),
        })
    return in_maps


def kernel(x, Wq, Wk, Wv, Wo, bo):
    from concourse.bass_utils import run_bass_kernel_spmd

    if "nc" not in _NC_CACHE:
        _NC_CACHE["nc"] = build_nc()
    nc = _NC_CACHE["nc"]

    in_maps = build_in_maps(x, Wq, Wk, Wv, Wo)
    res = run_bass_kernel_spmd(nc, in_maps, core_ids=list(range(8)))
    outs = [res.results[c]["out"].astype(np.float32) for c in range(8)]
    full = np.stack([outs[2 * b] + outs[2 * b + 1] for b in range(B)], axis=0)
    return (full + np.asarray(bo)[None, None, :].astype(np.float32)).astype(np.float32)


# revision 8
# speedup vs baseline: 1.1833x; 1.1833x over previous
"""Trainium2 Bass kernel for bucketed causal linear self-attention.

Model (B=4, T=4096, DIM=1024, H=16 heads, E=64, BUCKET=64):
  q,k,v = x@Wq, x@Wk, x@Wv ; q softmaxed over head-dim, k -> elu(k)+1
  per-bucket context C_u = cumsum_u(k_bu^T v_bu), normalized by cumsum of
  key-sums, shifted one bucket; attn_bu = q_bu @ C_{u-1}; out = attn@Wo + bo.

Sharding over 8 cores: core c -> batch c//2, head-group c%2 (8 heads = 512
feats). q/k/v column-sharded by head, Wo row-sharded; host sums the two
partial outputs per batch (all-reduce on host) and adds bo.

v5 structure per core:
  x arrives HOST-TRANSPOSED [DIM, T]; x^T tiles load as plain per-kt DMAs
  on the sync HWDGE ring.  attn^T accumulates on the tensor engine:
      C_carry^T @ q2  +  sum_j S_j^T @ q2[:, buckets > j]
  (shrinking-N matmuls; no per-bucket DVE walk).  psi = elu(k)+1 =
  min(exp(k),1) + relu(k): two ACT ops off the PSUM + one DVE stt, so the
  projection PSUM recycles fast.  Chunk emission is software-pipelined:
      q-proj(c) | attn-mms(c-1) | kv-proj+S+scan-chains(c) | out-proj(c-1)
  so every cross-engine chain resolves under dense unrelated PE work.
  Softmax reciprocal: one DVE reciprocal per chunk; per-token broadcast
  via a 2-row matmul.  q'' = exp(q)*softmax_recip*1/(ksum_prefix+eps),
  bucket-0 blindspot via a zeroed scale column on chunk 0.  Output bf16;
  host sums the two per-batch partials in f32 and adds bo.
"""

import sys
import numpy as np
import ml_dtypes

sys.path.insert(0, "/opt/trn_rl_repo")

B, T, DIM, H, BUCKET = 4, 4096, 1024, 16, 64
E = 64           # head dim
HC = 8           # heads per core
F = HC * E       # per-core feature width = 512
CH = 512         # tokens per chunk
UC = CH // BUCKET  # buckets per chunk = 8
PAIRS = HC // 2  # head pairs = 4
KT = DIM // 128  # contraction tiles = 8
EPS = 1e-6

_NC_CACHE = {}


def build_nc(n_chunks=T // CH):
    import concourse.bass as bass_mod
    import concourse.mybir as mybir
    from concourse import bacc
    from concourse.tile import TileContext

    BF16 = mybir.dt.bfloat16
    F32 = mybir.dt.float32
    AF = mybir.ActivationFunctionType
    OP = mybir.AluOpType

    Tt = n_chunks * CH

    nc = bacc.Bacc("TRN2", target_bir_lowering=False, debug=False, num_devices=8)
    x = nc.dram_tensor("x", [DIM, Tt], BF16, kind="ExternalInput").ap()
    wq = nc.dram_tensor("wq", [DIM, F], BF16, kind="ExternalInput").ap()
    wk = nc.dram_tensor("wk", [DIM, F], BF16, kind="ExternalInput").ap()
    wv = nc.dram_tensor("wv", [DIM, F], BF16, kind="ExternalInput").ap()
    wo = nc.dram_tensor("wo", [F, DIM], BF16, kind="ExternalInput").ap()
    orp = nc.dram_tensor("orp", [128, 128], BF16, kind="ExternalInput").ap()
    out = nc.dram_tensor("out", [Tt, DIM], BF16, kind="ExternalOutput").ap()

    with TileContext(nc) as tc:
        with tc.tile_pool(name="const", bufs=1) as constp, \
             tc.tile_pool(name="xt", bufs=2) as xtp, \
             tc.tile_pool(name="act", bufs=2) as actp, \
             tc.tile_pool(name="tmp", bufs=3) as tmpp, \
             tc.tile_pool(name="small", bufs=8) as smallp, \
             tc.tile_pool(name="outp", bufs=2) as outp, \
             tc.tile_pool(name="ps_proj", bufs=2, space="PSUM") as psproj, \
             tc.tile_pool(name="ps_s", bufs=1, space="PSUM") as pss, \
             tc.tile_pool(name="ps_attn", bufs=2, space="PSUM") as psattn, \
             tc.tile_pool(name="ps_misc", bufs=2, space="PSUM") as psmisc:

            # ---- resident constants ----
            wq_sb = constp.tile([128, KT, F], BF16, tag="wq")
            wk_sb = constp.tile([128, KT, F], BF16, tag="wk")
            wv_sb = constp.tile([128, KT, F], BF16, tag="wv")
            wo_sb = constp.tile([128, PAIRS, DIM], BF16, tag="wo")
            nc.gpsimd.dma_start(out=wq_sb[:], in_=wq.rearrange("(kt p) f -> p kt f", p=128))
            nc.gpsimd.dma_start(out=wk_sb[:], in_=wk.rearrange("(kt p) f -> p kt f", p=128))
            nc.gpsimd.dma_start(out=wv_sb[:], in_=wv.rearrange("(kt p) f -> p kt f", p=128))
            nc.gpsimd.dma_start(out=wo_sb[:], in_=wo.rearrange("(ft p) n -> p ft n", p=128))

            ones_sum = constp.tile([128, 2], BF16, tag="ones_sum")
            nc.vector.memset(ones_sum[:], 0.0)
            nc.vector.memset(ones_sum[0:64, 0:1], 1.0)
            nc.vector.memset(ones_sum[64:128, 1:2], 1.0)
            # orp[32p+0, 0:64]=1, orp[32p+1, 64:128]=1 (host-built)
            orp_sb = constp.tile([128, 128], BF16, tag="orp")
            nc.gpsimd.dma_start(out=orp_sb[:], in_=orp[:])

            # running context (+ ksum col 64) per pair, f32 master + bf16 copy
            c_ms = constp.tile([128, PAIRS, E + 1], F32, tag="c_ms")
            nc.vector.memset(c_ms[:], 0.0)
            c_bf = constp.tile([128, PAIRS, E], BF16, tag="c_bf")
            nc.vector.memset(c_bf[:], 0.0)

            state = {}

            def emit_q(c):
                xT = xtp.tile([128, KT, CH], BF16, tag="xT")
                for kt in range(KT):
                    nc.sync.dma_start(
                        out=xT[:, kt, :],
                        in_=x[kt * 128:(kt + 1) * 128, c * CH:(c + 1) * CH])

                # q^T, exp(q), per-token softmax sums (rows 32p..32p+2 of sm)
                E_sb = actp.tile([128, PAIRS, CH], BF16, tag="E")
                sm = psmisc.tile([128, CH], F32, tag="misc")
                for p in range(PAIRS):
                    qt = psproj.tile([128, CH], F32, tag="proj")
                    for kt in range(KT):
                        nc.tensor.matmul(qt[:], wq_sb[:, kt, p * 128:(p + 1) * 128],
                                         xT[:, kt, :], start=(kt == 0), stop=(kt == KT - 1))
                    nc.scalar.activation(out=E_sb[:, p, :], in_=qt[:], func=AF.Exp)
                    nc.tensor.matmul(sm[32 * p:32 * p + 2, :], ones_sum[:], E_sb[:, p, :],
                                     start=True, stop=True, tile_position=(0, 32 * p))
                recip_sb = actp.tile([128, CH], BF16, tag="recip")
                with nc.allow_low_precision(reason="bf16 softmax recip, 4e-3 rel"):
                    nc.vector.reciprocal(out=recip_sb[0:98, :], in_=sm[0:98, :])
                state[c] = {"xT": xT, "E": E_sb, "recip": recip_sb}

            def emit_kv(c):
                st = state[c]
                xT, E_sb, recip_sb = st["xT"], st["E"], st["recip"]
                psik = actp.tile([128, PAIRS, CH], BF16, tag="psik")
                v_sb = actp.tile([128, PAIRS, HC * (E + 1) // PAIRS * PAIRS], BF16, tag="v")
                # v_sb free layout per tok-tile: 8 heads x 65 (64 v + ones col)
                for tt in range(PAIRS):  # 4 token tiles of 128
                    kp = psproj.tile([128, F], F32, tag="proj")
                    for kt in range(KT):
                        nc.tensor.matmul(kp[:], xT[:, kt, tt * 128:(tt + 1) * 128],
                                         wk_sb[:, kt, :], start=(kt == 0), stop=(kt == KT - 1))
                    # psi = elu(k)+1 = min(exp(k),1) + relu(k); both ACT ops
                    # read the PSUM so it recycles fast
                    ek = tmpp.tile([128, F], BF16, tag="ek")
                    nc.scalar.activation(out=ek[:], in_=kp[:], func=AF.Exp)
                    rk = tmpp.tile([128, F], BF16, tag="rk")
                    nc.scalar.activation(out=rk[:], in_=kp[:], func=AF.Relu)
                    nc.vector.scalar_tensor_tensor(
                        out=psik[:, tt, :], in0=ek[:], scalar=1.0, in1=rk[:],
                        op0=OP.min, op1=OP.add)

                    vp = psproj.tile([128, F], F32, tag="proj")
                    for kt in range(KT):
                        nc.tensor.matmul(vp[:], xT[:, kt, tt * 128:(tt + 1) * 128],
                                         wv_sb[:, kt, :], start=(kt == 0), stop=(kt == KT - 1))
                    v3 = v_sb[:, tt, :].rearrange("p (h e1) -> p h e1", e1=E + 1)
                    nc.scalar.activation(
                        out=v3[:, :, 0:E],
                        in_=vp[:].rearrange("p (h e) -> p h e", e=E), func=AF.Copy)
                    nc.vector.memset(v3[:, :, E:E + 1], 1.0)

                # per-bucket context matmuls + scan chains, all pairs
                for p in range(PAIRS):
                    # S_j = psi_bu^T @ [v_bu | 1]
                    s_ev = pss.tile([128, UC // 2, E + 1], F32, tag="s_ev")
                    s_od = pss.tile([128, UC // 2, E + 1], F32, tag="s_od")
                    for j in range(UC):
                        sdst = s_ev if j % 2 == 0 else s_od
                        tt, r0 = j // 2, (j % 2) * 64
                        for hh in range(2):
                            h = 2 * p + hh
                            nc.tensor.matmul(
                                sdst[hh * 64:(hh + 1) * 64, j // 2, :],
                                psik[r0:r0 + 64, tt, h * E:(h + 1) * E],
                                v_sb[r0:r0 + 64, tt, :].rearrange(
                                    "p (g e1) -> p g e1", e1=E + 1)[:, h, :],
                                start=True, stop=True,
                                tile_position=(r0, hh * 64))
                    # q'' broadcast matmul can go right behind the S block
                    repl = psmisc.tile([128, CH], F32, tag="misc")
                    nc.tensor.matmul(repl[:], orp_sb[32 * p:32 * p + 2, :],
                                     recip_sb[32 * p:32 * p + 2, :],
                                     start=True, stop=True, tile_position=(32 * p, 0))

                    # all 8 buckets -> bf16 [128, (a 4) (b 2) 65]; b = j parity
                    s_bf = tmpp.tile([128, UC // 2, 2, E + 1], BF16, tag=f"s_bf{p}")
                    nc.vector.tensor_copy(out=s_bf[:, :, 0, :], in_=s_ev[:])
                    nc.vector.tensor_copy(out=s_bf[:, :, 1, :], in_=s_od[:])

                    # ksum exclusive prefix along buckets -> per-bucket scale R
                    ksv = s_bf[:].rearrange("p a b e1 -> p (a b) e1")[:, :, E:E + 1] \
                        .rearrange("p j one -> p (j one)")
                    ksc = smallp.tile([128, UC], F32, tag=f"ksc{p}")
                    nc.vector.tensor_tensor_scan(
                        out=ksc[:], data0=ksv, data1=ksv,
                        initial=c_ms[:, p, E:E + 1], op0=OP.add, op1=OP.bypass)
                    rs = smallp.tile([128, UC], F32, tag="rs")
                    nc.vector.tensor_copy(out=rs[:, 1:UC], in_=ksc[:, 0:UC - 1])
                    nc.vector.tensor_copy(out=rs[:, 0:1], in_=c_ms[:, p, E:E + 1])
                    R = smallp.tile([128, UC], BF16, tag="R")
                    nc.vector.tensor_scalar_add(rs[:], rs[:], EPS)
                    with nc.allow_low_precision(reason="bf16 ksum recip, 4e-3 rel"):
                        nc.vector.reciprocal(out=R[:], in_=rs[:])
                    if c == 0:
                        nc.vector.memset(R[:, 0:1], 0.0)  # bucket-0 blindspot

                    # q'' = exp(q) * softmax_recip * ksum_recip
                    rap = R[:]
                    Rb = bass_mod.AP(tensor=rap.tensor, offset=rap.offset,
                                     ap=[rap.ap[0], rap.ap[1], [0, BUCKET]])
                    RR = tmpp.tile([128, CH], BF16, tag="RR")
                    nc.vector.tensor_tensor(
                        out=RR[:].rearrange("p (u t) -> p u t", t=BUCKET),
                        in0=repl[:].rearrange("p (u t) -> p u t", t=BUCKET),
                        in1=Rb, op=OP.mult)
                    q2 = tmpp.tile([128, CH], BF16, tag=f"q2{p}")
                    nc.vector.tensor_tensor(out=q2[:], in0=E_sb[:, p, :], in1=RR[:],
                                            op=OP.mult)
                    st[f"s_bf{p}"] = s_bf
                    st[f"ksc{p}"] = ksc
                    st[f"q2{p}"] = q2

            def emit_attn(c):
                # attn^T accumulation per pair; the q2/scan chains were
                # resolved a whole q-projection ago
                st = state[c]
                atn = actp.tile([128, PAIRS, CH], BF16, tag="atn")
                st["atn"] = atn
                for p in range(PAIRS):
                    s_bf, ksc, q2 = st[f"s_bf{p}"], st[f"ksc{p}"], st[f"q2{p}"]
                    at = psattn.tile([128, CH], F32, tag="attn")
                    for hh in range(2):
                        r0 = hh * 64
                        nc.tensor.matmul(
                            at[r0:r0 + 64, :], c_bf[r0:r0 + 64, p, :],
                            q2[r0:r0 + 64, :], start=True, stop=False,
                            tile_position=(r0, r0))
                        for j in range(UC - 1):
                            q0 = (j + 1) * BUCKET
                            nc.tensor.matmul(
                                at[r0:r0 + 64, q0:CH],
                                s_bf[r0:r0 + 64, j // 2, j % 2, 0:E],
                                q2[r0:r0 + 64, q0:CH], start=False,
                                stop=(j == UC - 2), tile_position=(r0, r0))
                    nc.vector.tensor_copy(out=atn[:, p, :], in_=at[:])

                    # advance the running carry (reads above see the old state)
                    c_red = smallp.tile([128, E], F32, tag="c_red")
                    nc.vector.tensor_reduce(
                        out=c_red[:],
                        in_=s_bf[:].rearrange("p a b e1 -> p e1 (a b)")[:, 0:E, :],
                        axis=mybir.AxisListType.X, op=OP.add)
                    nc.vector.tensor_tensor(out=c_ms[:, p, 0:E], in0=c_ms[:, p, 0:E],
                                            in1=c_red[:], op=OP.add)
                    nc.vector.tensor_copy(out=c_ms[:, p, E:E + 1], in_=ksc[:, UC - 1:UC])
                    nc.vector.tensor_copy(out=c_bf[:, p, :], in_=c_ms[:, p, 0:E])

            def emit_out(c):
                st = state.pop(c)
                atn = st["atn"]
                osb = outp.tile([128, PAIRS, DIM], BF16, tag="osb")
                for tt in range(PAIRS):
                    for half in range(2):
                        op_ = psproj.tile([128, 512], F32, tag="proj")
                        for p in range(PAIRS):
                            nc.tensor.matmul(
                                op_[:], atn[:, p, tt * 128:(tt + 1) * 128],
                                wo_sb[:, p, half * 512:(half + 1) * 512],
                                start=(p == 0), stop=(p == PAIRS - 1))
                        nc.scalar.activation(
                            out=osb[:, tt, half * 512:(half + 1) * 512],
                            in_=op_[:], func=AF.Copy)
                nc.gpsimd.dma_start(
                    out=out[c * CH:(c + 1) * CH, :].rearrange(
                        "(tt p) d -> p tt d", p=128),
                    in_=osb[:])

            for c in range(n_chunks):
                emit_q(c)
                if c >= 1:
                    emit_attn(c - 1)
                emit_kv(c)
                if c >= 1:
                    emit_out(c - 1)
            emit_attn(n_chunks - 1)
            emit_out(n_chunks - 1)

    nc.finalize()
    return nc


def _orp():
    m = np.zeros((128, 128), dtype=ml_dtypes.bfloat16)
    for p in range(PAIRS):
        m[32 * p, 0:64] = 1
        m[32 * p + 1, 64:128] = 1
    return m


def build_in_maps(x, Wq, Wk, Wv, Wo):
    bf = ml_dtypes.bfloat16
    x = np.asarray(x)
    Wq, Wk, Wv, Wo = (np.asarray(w) for w in (Wq, Wk, Wv, Wo))
    in_maps = []
    for c in range(8):
        b, g = c // 2, c % 2
        sl = slice(g * F, (g + 1) * F)
        in_maps.append({
            "x": np.ascontiguousarray(x[b].T).astype(bf),
            "wq": Wq[:, sl].astype(bf),
            "wk": Wk[:, sl].astype(bf),
            "wv": Wv[:, sl].astype(bf),
            "wo": Wo[sl, :].astype(bf),
            "orp": _orp(),
        })
    return in_maps


def kernel(x, Wq, Wk, Wv, Wo, bo):
    from concourse.bass_utils import run_bass_kernel_spmd

    if "nc" not in _NC_CACHE:
        _NC_CACHE["nc"] = build_nc()
    nc = _NC_CACHE["nc"]

    in_maps = build_in_maps(x, Wq, Wk, Wv, Wo)
    res = run_bass_kernel_spmd(nc, in_maps, core_ids=list(range(8)))
    outs = [res.results[c]["out"].astype(np.float32) for c in range(8)]
    full = np.stack([outs[2 * b] + outs[2 * b + 1] for b in range(B)], axis=0)
    return (full + np.asarray(bo)[None, None, :].astype(np.float32)).astype(np.float32)


# revision 12
# speedup vs baseline: 1.3257x; 1.1203x over previous
"""Trainium2 Bass kernel for bucketed causal linear self-attention.

Model (B=4, T=4096, DIM=1024, H=16 heads, E=64, BUCKET=64):
  q,k,v = x@Wq, x@Wk, x@Wv ; q softmaxed over head-dim, k -> elu(k)+1
  per-bucket context C_u = cumsum_u(k_bu^T v_bu), normalized by cumsum of
  key-sums, shifted one bucket; attn_bu = q_bu @ C_{u-1}; out = attn@Wo + bo.

Sharding over 8 cores: core c -> batch c//2, head-group c%2 (8 heads = 512
feats). q/k/v column-sharded by head, Wo row-sharded; host sums the two
partial outputs per batch (all-reduce on host) and adds bo.

v5 structure per core:
  x arrives HOST-TRANSPOSED [DIM, T]; x^T tiles load as plain per-kt DMAs
  on the sync HWDGE ring.  attn^T accumulates on the tensor engine:
      C_carry^T @ q2  +  sum_j S_j^T @ q2[:, buckets > j]
  (shrinking-N matmuls; no per-bucket DVE walk).  psi = elu(k)+1 =
  min(exp(k),1) + relu(k): two ACT ops off the PSUM + one DVE stt, so the
  projection PSUM recycles fast.  Chunk emission is software-pipelined:
      q-proj(c) | attn-mms(c-1) | kv-proj+S+scan-chains(c) | out-proj(c-1)
  so every cross-engine chain resolves under dense unrelated PE work.
  Softmax reciprocal: one DVE reciprocal per chunk; per-token broadcast
  via a 2-row matmul.  q'' = exp(q)*softmax_recip*1/(ksum_prefix+eps),
  bucket-0 blindspot via a zeroed scale column on chunk 0.  Output bf16;
  host sums the two per-batch partials in f32 and adds bo.
"""

import sys
import numpy as np
import ml_dtypes

sys.path.insert(0, "/opt/trn_rl_repo")

B, T, DIM, H, BUCKET = 4, 4096, 1024, 16, 64
E = 64           # head dim
HC = 8           # heads per core
F = HC * E       # per-core feature width = 512
CH = 512         # tokens per chunk
UC = CH // BUCKET  # buckets per chunk = 8
PAIRS = HC // 2  # head pairs = 4
KT = DIM // 128  # contraction tiles = 8
EPS = 1e-6

_NC_CACHE = {}


def build_nc(n_chunks=T // CH):
    import concourse.bass as bass_mod
    import concourse.mybir as mybir
    from concourse import bacc
    from concourse.tile import TileContext

    BF16 = mybir.dt.bfloat16
    F32 = mybir.dt.float32
    AF = mybir.ActivationFunctionType
    OP = mybir.AluOpType

    Tt = n_chunks * CH

    nc = bacc.Bacc("TRN2", target_bir_lowering=False, debug=False, num_devices=8)
    x = nc.dram_tensor("x", [DIM, Tt], BF16, kind="ExternalInput").ap()
    wq = nc.dram_tensor("wq", [DIM, F], BF16, kind="ExternalInput").ap()
    wk = nc.dram_tensor("wk", [DIM, F], BF16, kind="ExternalInput").ap()
    wv = nc.dram_tensor("wv", [DIM, F], BF16, kind="ExternalInput").ap()
    wo = nc.dram_tensor("wo", [F, DIM], BF16, kind="ExternalInput").ap()
    orp = nc.dram_tensor("orp", [128, 128], BF16, kind="ExternalInput").ap()
    out = nc.dram_tensor("out", [Tt, DIM], BF16, kind="ExternalOutput").ap()

    with TileContext(nc) as tc:
        with tc.tile_pool(name="const", bufs=1) as constp, \
             tc.tile_pool(name="xt", bufs=2) as xtp, \
             tc.tile_pool(name="act", bufs=2) as actp, \
             tc.tile_pool(name="tmp", bufs=3) as tmpp, \
             tc.tile_pool(name="small", bufs=8) as smallp, \
             tc.tile_pool(name="outp", bufs=2) as outp, \
             tc.tile_pool(name="ps_proj", bufs=2, space="PSUM") as psproj, \
             tc.tile_pool(name="ps_s", bufs=1, space="PSUM") as pss, \
             tc.tile_pool(name="ps_attn", bufs=2, space="PSUM") as psattn, \
             tc.tile_pool(name="ps_misc", bufs=2, space="PSUM") as psmisc:

            # ---- resident constants ----
            wq_sb = constp.tile([128, KT, F], BF16, tag="wq")
            wk_sb = constp.tile([128, KT, F], BF16, tag="wk")
            wv_sb = constp.tile([128, KT, F], BF16, tag="wv")
            wo_sb = constp.tile([128, PAIRS, DIM], BF16, tag="wo")
            # wq on the scalar HWDGE ring so it lands in parallel with x^T
            # (sync ring) and the rest (gpsimd ring)
            nc.scalar.dma_start(out=wq_sb[:], in_=wq.rearrange("(kt p) f -> p kt f", p=128))
            nc.gpsimd.dma_start(out=wk_sb[:], in_=wk.rearrange("(kt p) f -> p kt f", p=128))
            nc.gpsimd.dma_start(out=wv_sb[:], in_=wv.rearrange("(kt p) f -> p kt f", p=128))
            nc.gpsimd.dma_start(out=wo_sb[:], in_=wo.rearrange("(ft p) n -> p ft n", p=128))

            ones_sum = constp.tile([128, 2], BF16, tag="ones_sum")
            nc.vector.memset(ones_sum[:], 0.0)
            nc.vector.memset(ones_sum[0:64, 0:1], 1.0)
            nc.vector.memset(ones_sum[64:128, 1:2], 1.0)
            # orp[32p+0, 0:64]=1, orp[32p+1, 64:128]=1 (host-built)
            orp_sb = constp.tile([128, 128], BF16, tag="orp")
            nc.gpsimd.dma_start(out=orp_sb[:], in_=orp[:])

            # running context (+ ksum col 64) per pair, f32 master + bf16 copy
            c_ms = constp.tile([128, PAIRS, E + 1], F32, tag="c_ms")
            nc.vector.memset(c_ms[:], 0.0)
            c_bf = constp.tile([128, PAIRS, E], BF16, tag="c_bf")
            nc.vector.memset(c_bf[:], 0.0)

            state = {}

            def emit_q(c):
                xT = xtp.tile([128, KT, CH], BF16, tag="xT")
                for kt in range(KT):
                    nc.sync.dma_start(
                        out=xT[:, kt, :],
                        in_=x[kt * 128:(kt + 1) * 128, c * CH:(c + 1) * CH])

                # q^T, exp(q), per-token softmax sums (rows 32p..32p+2 of sm)
                E_sb = actp.tile([128, PAIRS, CH], BF16, tag="E")
                sm = psmisc.tile([128, CH], F32, tag="misc")
                for p in range(PAIRS):
                    qt = psproj.tile([128, CH], F32, tag="proj")
                    for kt in range(KT):
                        nc.tensor.matmul(qt[:], wq_sb[:, kt, p * 128:(p + 1) * 128],
                                         xT[:, kt, :], start=(kt == 0), stop=(kt == KT - 1))
                    nc.scalar.activation(out=E_sb[:, p, :], in_=qt[:], func=AF.Exp)
                    nc.tensor.matmul(sm[32 * p:32 * p + 2, :], ones_sum[:], E_sb[:, p, :],
                                     start=True, stop=True, tile_position=(0, 32 * p))
                state[c] = {"xT": xT, "E": E_sb, "sm": sm}

            def emit_kv(c):
                st = state[c]
                xT, E_sb, sm = st["xT"], st["E"], st.pop("sm")
                # softmax reciprocal emitted here so its 3.4us DVE occupancy
                # queues after the previous chunk's attn-phase casts
                recip_sb = actp.tile([128, CH], BF16, tag="recip")
                with nc.allow_low_precision(reason="bf16 softmax recip, 4e-3 rel"):
                    nc.vector.reciprocal(out=recip_sb[0:98, :], in_=sm[0:98, :])
                psik = actp.tile([128, PAIRS, CH], BF16, tag="psik")
                v_sb = actp.tile([128, PAIRS, HC * (E + 1) // PAIRS * PAIRS], BF16, tag="v")
                # v_sb free layout per tok-tile: 8 heads x 65 (64 v + ones col)
                for tt in range(PAIRS):  # 4 token tiles of 128
                    kp = psproj.tile([128, F], F32, tag="proj")
                    for kt in range(KT):
                        nc.tensor.matmul(kp[:], xT[:, kt, tt * 128:(tt + 1) * 128],
                                         wk_sb[:, kt, :], start=(kt == 0), stop=(kt == KT - 1))
                    # psi = elu(k)+1 = min(exp(k),1) + relu(k); both ACT ops
                    # read the PSUM so it recycles fast
                    ek = tmpp.tile([128, F], BF16, tag="ek")
                    nc.scalar.activation(out=ek[:], in_=kp[:], func=AF.Exp)
                    rk = tmpp.tile([128, F], BF16, tag="rk")
                    nc.scalar.activation(out=rk[:], in_=kp[:], func=AF.Relu)
                    nc.vector.scalar_tensor_tensor(
                        out=psik[:, tt, :], in0=ek[:], scalar=1.0, in1=rk[:],
                        op0=OP.min, op1=OP.add)

                    vp = psproj.tile([128, F], F32, tag="proj")
                    for kt in range(KT):
                        nc.tensor.matmul(vp[:], xT[:, kt, tt * 128:(tt + 1) * 128],
                                         wv_sb[:, kt, :], start=(kt == 0), stop=(kt == KT - 1))
                    v3 = v_sb[:, tt, :].rearrange("p (h e1) -> p h e1", e1=E + 1)
                    nc.scalar.activation(
                        out=v3[:, :, 0:E],
                        in_=vp[:].rearrange("p (h e) -> p h e", e=E), func=AF.Copy)
                    nc.vector.memset(v3[:, :, E:E + 1], 1.0)

                # per-bucket context matmuls + scan chains, all pairs
                for p in range(PAIRS):
                    # S_j = psi_bu^T @ [v_bu | 1]
                    s_ev = pss.tile([128, UC // 2, E + 1], F32, tag="s_ev")
                    s_od = pss.tile([128, UC // 2, E + 1], F32, tag="s_od")
                    for j in range(UC):
                        sdst = s_ev if j % 2 == 0 else s_od
                        tt, r0 = j // 2, (j % 2) * 64
                        for hh in range(2):
                            h = 2 * p + hh
                            nc.tensor.matmul(
                                sdst[hh * 64:(hh + 1) * 64, j // 2, :],
                                psik[r0:r0 + 64, tt, h * E:(h + 1) * E],
                                v_sb[r0:r0 + 64, tt, :].rearrange(
                                    "p (g e1) -> p g e1", e1=E + 1)[:, h, :],
                                start=True, stop=True,
                                tile_position=(r0, hh * 64))
                    # q'' broadcast matmul can go right behind the S block
                    repl = psmisc.tile([128, CH], F32, tag="misc")
                    nc.tensor.matmul(repl[:], orp_sb[32 * p:32 * p + 2, :],
                                     recip_sb[32 * p:32 * p + 2, :],
                                     start=True, stop=True, tile_position=(32 * p, 0))

                    # all 8 buckets -> bf16 [128, (a 4) (b 2) 65]; b = j parity
                    s_bf = tmpp.tile([128, UC // 2, 2, E + 1], BF16, tag=f"s_bf{p}")
                    nc.vector.tensor_copy(out=s_bf[:, :, 0, :], in_=s_ev[:])
                    nc.vector.tensor_copy(out=s_bf[:, :, 1, :], in_=s_od[:])

                    # ksum exclusive prefix along buckets -> per-bucket scale R
                    ksv = s_bf[:].rearrange("p a b e1 -> p (a b) e1")[:, :, E:E + 1] \
                        .rearrange("p j one -> p (j one)")
                    ksc = smallp.tile([128, UC], F32, tag=f"ksc{p}")
                    nc.vector.tensor_tensor_scan(
                        out=ksc[:], data0=ksv, data1=ksv,
                        initial=c_ms[:, p, E:E + 1], op0=OP.add, op1=OP.bypass)
                    rs = smallp.tile([128, UC], F32, tag="rs")
                    nc.vector.tensor_copy(out=rs[:, 1:UC], in_=ksc[:, 0:UC - 1])
                    nc.vector.tensor_copy(out=rs[:, 0:1], in_=c_ms[:, p, E:E + 1])
                    R = smallp.tile([128, UC], BF16, tag="R")
                    nc.vector.tensor_scalar_add(rs[:], rs[:], EPS)
                    with nc.allow_low_precision(reason="bf16 ksum recip, 4e-3 rel"):
                        nc.vector.reciprocal(out=R[:], in_=rs[:])
                    if c == 0:
                        nc.vector.memset(R[:, 0:1], 0.0)  # bucket-0 blindspot

                    # q'' = exp(q) * softmax_recip * ksum_recip
                    rap = R[:]
                    Rb = bass_mod.AP(tensor=rap.tensor, offset=rap.offset,
                                     ap=[rap.ap[0], rap.ap[1], [0, BUCKET]])
                    RR = tmpp.tile([128, CH], BF16, tag="RR")
                    nc.vector.tensor_tensor(
                        out=RR[:].rearrange("p (u t) -> p u t", t=BUCKET),
                        in0=repl[:].rearrange("p (u t) -> p u t", t=BUCKET),
                        in1=Rb, op=OP.mult)
                    q2 = tmpp.tile([128, CH], BF16, tag=f"q2{p}")
                    nc.vector.tensor_tensor(out=q2[:], in0=E_sb[:, p, :], in1=RR[:],
                                            op=OP.mult)
                    st[f"s_bf{p}"] = s_bf
                    st[f"ksc{p}"] = ksc
                    st[f"q2{p}"] = q2

            def emit_attn(c):
                # attn^T accumulation per pair; the q2/scan chains were
                # resolved a whole q-projection ago
                st = state[c]
                atn = actp.tile([128, PAIRS, CH], BF16, tag="atn")
                st["atn"] = atn
                for p in range(PAIRS):
                    s_bf, ksc, q2 = st[f"s_bf{p}"], st[f"ksc{p}"], st[f"q2{p}"]
                    at = psattn.tile([128, CH], F32, tag="attn")
                    for hh in range(2):
                        r0 = hh * 64
                        nc.tensor.matmul(
                            at[r0:r0 + 64, :], c_bf[r0:r0 + 64, p, :],
                            q2[r0:r0 + 64, :], start=True, stop=False,
                            tile_position=(r0, r0))
                        for j in range(UC - 1):
                            q0 = (j + 1) * BUCKET
                            nc.tensor.matmul(
                                at[r0:r0 + 64, q0:CH],
                                s_bf[r0:r0 + 64, j // 2, j % 2, 0:E],
                                q2[r0:r0 + 64, q0:CH], start=False,
                                stop=(j == UC - 2), tile_position=(r0, r0))
                    nc.scalar.activation(out=atn[:, p, :], in_=at[:], func=AF.Copy)

                    # advance the running carry (reads above see the old state)
                    c_red = smallp.tile([128, E], F32, tag="c_red")
                    nc.vector.tensor_reduce(
                        out=c_red[:],
                        in_=s_bf[:].rearrange("p a b e1 -> p e1 (a b)")[:, 0:E, :],
                        axis=mybir.AxisListType.X, op=OP.add)
                    nc.vector.tensor_tensor(out=c_ms[:, p, 0:E], in0=c_ms[:, p, 0:E],
                                            in1=c_red[:], op=OP.add)
                    nc.vector.tensor_copy(out=c_ms[:, p, E:E + 1], in_=ksc[:, UC - 1:UC])
                    nc.vector.tensor_copy(out=c_bf[:, p, :], in_=c_ms[:, p, 0:E])

            def emit_out(c):
                st = state.pop(c)
                atn = st["atn"]
                osb = outp.tile([128, PAIRS, DIM], BF16, tag="osb")
                for tt in range(PAIRS):
                    for half in range(2):
                        op_ = psproj.tile([128, 512], F32, tag="proj")
                        for p in range(PAIRS):
                            nc.tensor.matmul(
                                op_[:], atn[:, p, tt * 128:(tt + 1) * 128],
                                wo_sb[:, p, half * 512:(half + 1) * 512],
                                start=(p == 0), stop=(p == PAIRS - 1))
                        nc.scalar.activation(
                            out=osb[:, tt, half * 512:(half + 1) * 512],
                            in_=op_[:], func=AF.Copy)
                nc.gpsimd.dma_start(
                    out=out[c * CH:(c + 1) * CH, :].rearrange(
                        "(tt p) d -> p tt d", p=128),
                    in_=osb[:])

            for c in range(n_chunks):
                emit_q(c)
                if c >= 1:
                    emit_attn(c - 1)
                emit_kv(c)
                if c >= 1:
                    emit_out(c - 1)
            emit_attn(n_chunks - 1)
            emit_out(n_chunks - 1)

    nc.finalize()
    return nc


def _orp():
    m = np.zeros((128, 128), dtype=ml_dtypes.bfloat16)
    for p in range(PAIRS):
        m[32 * p, 0:64] = 1
        m[32 * p + 1, 64:128] = 1
    return m


def build_in_maps(x, Wq, Wk, Wv, Wo):
    bf = ml_dtypes.bfloat16
    x = np.asarray(x)
    Wq, Wk, Wv, Wo = (np.asarray(w) for w in (Wq, Wk, Wv, Wo))
    in_maps = []
    for c in range(8):
        b, g = c // 2, c % 2
        sl = slice(g * F, (g + 1) * F)
        in_maps.append({
            "x": np.ascontiguousarray(x[b].T).astype(bf),
            "wq": Wq[:, sl].astype(bf),
            "wk": Wk[:, sl].astype(bf),
            "wv": Wv[:, sl].astype(bf),
            "wo": Wo[sl, :].astype(bf),
            "orp": _orp(),
        })
    return in_maps


def kernel(x, Wq, Wk, Wv, Wo, bo):
    from concourse.bass_utils import run_bass_kernel_spmd

    if "nc" not in _NC_CACHE:
        _NC_CACHE["nc"] = build_nc()
    nc = _NC_CACHE["nc"]

    in_maps = build_in_maps(x, Wq, Wk, Wv, Wo)
    res = run_bass_kernel_spmd(nc, in_maps, core_ids=list(range(8)))
    outs = [res.results[c]["out"].astype(np.float32) for c in range(8)]
    full = np.stack([outs[2 * b] + outs[2 * b + 1] for b in range(B)], axis=0)
    return (full + np.asarray(bo)[None, None, :].astype(np.float32)).astype(np.float32)


# revision 18
# speedup vs baseline: 1.3532x; 1.0208x over previous
"""Trainium2 Bass kernel for bucketed causal linear self-attention.

Model (B=4, T=4096, DIM=1024, H=16 heads, E=64, BUCKET=64):
  q,k,v = x@Wq, x@Wk, x@Wv ; q softmaxed over head-dim, k -> elu(k)+1
  per-bucket context C_u = cumsum_u(k_bu^T v_bu), normalized by cumsum of
  key-sums, shifted one bucket; attn_bu = q_bu @ C_{u-1}; out = attn@Wo + bo.

Sharding over 8 cores: core c -> batch c//2, head-group c%2 (8 heads = 512
feats). q/k/v column-sharded by head, Wo row-sharded; host sums the two
partial outputs per batch (all-reduce on host) and adds bo.

v5 structure per core:
  x arrives HOST-TRANSPOSED [DIM, T]; x^T tiles load as plain per-kt DMAs
  on the sync HWDGE ring.  attn^T accumulates on the tensor engine:
      C_carry^T @ q2  +  sum_j S_j^T @ q2[:, buckets > j]
  (shrinking-N matmuls; no per-bucket DVE walk).  psi = elu(k)+1 =
  min(exp(k),1) + relu(k): two ACT ops off the PSUM + one DVE stt, so the
  projection PSUM recycles fast.  Chunk emission is software-pipelined:
      q-proj(c) | attn-mms(c-1) | kv-proj+S+scan-chains(c) | out-proj(c-1)
  so every cross-engine chain resolves under dense unrelated PE work.
  Softmax reciprocal: one DVE reciprocal per chunk; per-token broadcast
  via a 2-row matmul.  q'' = exp(q)*softmax_recip*1/(ksum_prefix+eps),
  bucket-0 blindspot via a zeroed scale column on chunk 0.  Output bf16;
  host sums the two per-batch partials in f32 and adds bo.
"""

import sys
import numpy as np
import ml_dtypes

sys.path.insert(0, "/opt/trn_rl_repo")

B, T, DIM, H, BUCKET = 4, 4096, 1024, 16, 64
E = 64           # head dim
HC = 8           # heads per core
F = HC * E       # per-core feature width = 512
CH = 512         # tokens per chunk
UC = CH // BUCKET  # buckets per chunk = 8
PAIRS = HC // 2  # head pairs = 4
KT = DIM // 128  # contraction tiles = 8
EPS = 1e-6

_NC_CACHE = {}


def build_nc(n_chunks=T // CH):
    import concourse.bass as bass_mod
    import concourse.mybir as mybir
    from concourse import bacc
    from concourse.tile import TileContext

    BF16 = mybir.dt.bfloat16
    F32 = mybir.dt.float32
    AF = mybir.ActivationFunctionType
    OP = mybir.AluOpType

    Tt = n_chunks * CH

    nc = bacc.Bacc("TRN2", target_bir_lowering=False, debug=False, num_devices=8)
    x = nc.dram_tensor("x", [DIM, Tt], BF16, kind="ExternalInput").ap()
    wq = nc.dram_tensor("wq", [DIM, F], BF16, kind="ExternalInput").ap()
    wk = nc.dram_tensor("wk", [DIM, F], BF16, kind="ExternalInput").ap()
    wv = nc.dram_tensor("wv", [DIM, F], BF16, kind="ExternalInput").ap()
    wo = nc.dram_tensor("wo", [F, DIM], BF16, kind="ExternalInput").ap()
    orp = nc.dram_tensor("orp", [128, 128], BF16, kind="ExternalInput").ap()
    out = nc.dram_tensor("out", [Tt, DIM], BF16, kind="ExternalOutput").ap()

    with TileContext(nc) as tc:
        with tc.tile_pool(name="const", bufs=1) as constp, \
             tc.tile_pool(name="xt", bufs=2) as xtp, \
             tc.tile_pool(name="act", bufs=2) as actp, \
             tc.tile_pool(name="tmp", bufs=3) as tmpp, \
             tc.tile_pool(name="small", bufs=8) as smallp, \
             tc.tile_pool(name="outp", bufs=2) as outp, \
             tc.tile_pool(name="ps_proj", bufs=2, space="PSUM") as psproj, \
             tc.tile_pool(name="ps_s", bufs=1, space="PSUM") as pss, \
             tc.tile_pool(name="ps_attn", bufs=2, space="PSUM") as psattn, \
             tc.tile_pool(name="ps_misc", bufs=2, space="PSUM") as psmisc:

            # ---- resident constants ----
            wq_sb = constp.tile([128, KT, F], BF16, tag="wq")
            wk_sb = constp.tile([128, KT, F], BF16, tag="wk")
            wv_sb = constp.tile([128, KT, F], BF16, tag="wv")
            wo_sb = constp.tile([128, PAIRS, DIM], BF16, tag="wo")
            # wq on the scalar HWDGE ring so it lands in parallel with x^T
            # (sync ring) and the rest (gpsimd ring); per-kt pieces so the
            # first q matmul starts after 128KB, not 1MB
            for kt in range(KT):
                nc.scalar.dma_start(out=wq_sb[:, kt, :],
                                    in_=wq[kt * 128:(kt + 1) * 128, :])
            nc.gpsimd.dma_start(out=wk_sb[:], in_=wk.rearrange("(kt p) f -> p kt f", p=128))
            nc.gpsimd.dma_start(out=wv_sb[:], in_=wv.rearrange("(kt p) f -> p kt f", p=128))
            nc.gpsimd.dma_start(out=wo_sb[:], in_=wo.rearrange("(ft p) n -> p ft n", p=128))

            ones_sum = constp.tile([128, 2], BF16, tag="ones_sum")
            nc.vector.memset(ones_sum[:], 0.0)
            nc.vector.memset(ones_sum[0:64, 0:1], 1.0)
            nc.vector.memset(ones_sum[64:128, 1:2], 1.0)
            # orp[32p+0, 0:64]=1, orp[32p+1, 64:128]=1 (host-built)
            orp_sb = constp.tile([128, 128], BF16, tag="orp")
            nc.gpsimd.dma_start(out=orp_sb[:], in_=orp[:])

            # running context (+ ksum col 64) per pair, f32 master + a bf16
            # BLOCK-DIAGONAL copy (head A in rows 0:64 x cols 0:64, head B in
            # rows 64:128 x cols 64:128) so one matmul covers both heads
            c_ms = constp.tile([128, PAIRS, E + 1], F32, tag="c_ms")
            nc.vector.memset(c_ms[:], 0.0)
            c_bd = constp.tile([128, PAIRS, 128], BF16, tag="c_bd")
            nc.vector.memset(c_bd[:], 0.0)
            # per-bucket context in the same block-diagonal form (j = 0..6;
            # bucket 7 never feeds attn); zeros off-diagonal, set once
            s_bd = constp.tile([128, PAIRS, UC - 1, 128], BF16, tag="s_bd")
            nc.vector.memset(s_bd[:], 0.0)

            state = {}

            def emit_q(c):
                xT = xtp.tile([128, KT, CH], BF16, tag="xT")
                for kt in range(KT):
                    nc.sync.dma_start(
                        out=xT[:, kt, :],
                        in_=x[kt * 128:(kt + 1) * 128, c * CH:(c + 1) * CH])

                # q^T, exp(q), per-token softmax sums (rows 32p..32p+2 of sm)
                E_sb = actp.tile([128, PAIRS, CH], BF16, tag="E")
                sm = psmisc.tile([128, CH], F32, tag="misc")
                for p in range(PAIRS):
                    qt = psproj.tile([128, CH], F32, tag="proj")
                    for kt in range(KT):
                        nc.tensor.matmul(qt[:], wq_sb[:, kt, p * 128:(p + 1) * 128],
                                         xT[:, kt, :], start=(kt == 0), stop=(kt == KT - 1))
                    nc.scalar.activation(out=E_sb[:, p, :], in_=qt[:], func=AF.Exp)
                    nc.tensor.matmul(sm[32 * p:32 * p + 2, :], ones_sum[:], E_sb[:, p, :],
                                     start=True, stop=True, tile_position=(0, 32 * p))
                state[c] = {"xT": xT, "E": E_sb, "sm": sm}

            def emit_kv(c):
                st = state[c]
                xT, E_sb, sm = st["xT"], st["E"], st.pop("sm")
                # softmax reciprocal emitted here so its 3.4us DVE occupancy
                # queues after the previous chunk's attn-phase casts
                recip_sb = actp.tile([128, CH], BF16, tag="recip")
                with nc.allow_low_precision(reason="bf16 softmax recip, 4e-3 rel"):
                    nc.vector.reciprocal(out=recip_sb[0:98, :], in_=sm[0:98, :])
                psik = actp.tile([128, PAIRS, CH], BF16, tag="psik")
                v_sb = actp.tile([128, PAIRS, HC * (E + 1) // PAIRS * PAIRS], BF16, tag="v")
                # v_sb free layout per tok-tile: 8 heads x 65 (64 v + ones col)
                for tt in range(PAIRS):  # 4 token tiles of 128
                    kp = psproj.tile([128, F], F32, tag="proj")
                    for kt in range(KT):
                        nc.tensor.matmul(kp[:], xT[:, kt, tt * 128:(tt + 1) * 128],
                                         wk_sb[:, kt, :], start=(kt == 0), stop=(kt == KT - 1))
                    # psi = elu(k)+1 = min(exp(k),1) + relu(k); both ACT ops
                    # read the PSUM so it recycles fast
                    ek = tmpp.tile([128, F], BF16, tag="ek")
                    nc.scalar.activation(out=ek[:], in_=kp[:], func=AF.Exp)
                    rk = tmpp.tile([128, F], BF16, tag="rk")
                    nc.scalar.activation(out=rk[:], in_=kp[:], func=AF.Relu)
                    nc.vector.scalar_tensor_tensor(
                        out=psik[:, tt, :], in0=ek[:], scalar=1.0, in1=rk[:],
                        op0=OP.min, op1=OP.add)

                    vp = psproj.tile([128, F], F32, tag="proj")
                    for kt in range(KT):
                        nc.tensor.matmul(vp[:], xT[:, kt, tt * 128:(tt + 1) * 128],
                                         wv_sb[:, kt, :], start=(kt == 0), stop=(kt == KT - 1))
                    v3 = v_sb[:, tt, :].rearrange("p (h e1) -> p h e1", e1=E + 1)
                    nc.scalar.activation(
                        out=v3[:, :, 0:E],
                        in_=vp[:].rearrange("p (h e) -> p h e", e=E), func=AF.Copy)
                    nc.vector.memset(v3[:, :, E:E + 1], 1.0)

                # per-bucket context matmuls + scan chains, all pairs
                for p in range(PAIRS):
                    # S_j = psi_bu^T @ [v_bu | 1]
                    s_ev = pss.tile([128, UC // 2, E + 1], F32, tag="s_ev")
                    s_od = pss.tile([128, UC // 2, E + 1], F32, tag="s_od")
                    for j in range(UC):
                        sdst = s_ev if j % 2 == 0 else s_od
                        tt, r0 = j // 2, (j % 2) * 64
                        for hh in range(2):
                            h = 2 * p + hh
                            nc.tensor.matmul(
                                sdst[hh * 64:(hh + 1) * 64, j // 2, :],
                                psik[r0:r0 + 64, tt, h * E:(h + 1) * E],
                                v_sb[r0:r0 + 64, tt, :].rearrange(
                                    "p (g e1) -> p g e1", e1=E + 1)[:, h, :],
                                start=True, stop=True,
                                tile_position=(r0, hh * 64))
                    # q'' broadcast matmul can go right behind the S block
                    repl = psmisc.tile([128, CH], F32, tag="misc")
                    nc.tensor.matmul(repl[:], orp_sb[32 * p:32 * p + 2, :],
                                     recip_sb[32 * p:32 * p + 2, :],
                                     start=True, stop=True, tile_position=(32 * p, 0))

                    # S_j -> bf16 block-diagonal [128, j, 128]: head A rows
                    # 0:64 x cols 0:64, head B rows 64:128 x cols 64:128.
                    # j parity interleave via stepped-stride APs.
                    for hh in range(2):
                        r0, col0 = hh * 64, hh * 64
                        dev = s_bd[r0:r0 + 64, p, 0:UC - 1, col0:col0 + 64]
                        dev_s = bass_mod.AP(tensor=dev.tensor, offset=dev.offset,
                                            ap=[dev.ap[0], [2 * 128, 4], dev.ap[2]])
                        nc.vector.tensor_copy(out=dev_s, in_=s_ev[r0:r0 + 64, :, 0:E])
                        dod = s_bd[r0:r0 + 64, p, 1:UC - 1, col0:col0 + 64]
                        dod_s = bass_mod.AP(tensor=dod.tensor, offset=dod.offset,
                                            ap=[dod.ap[0], [2 * 128, 3], dod.ap[2]])
                        nc.vector.tensor_copy(out=dod_s, in_=s_od[r0:r0 + 64, 0:3, 0:E])

                    # per-bucket key sums -> [128, 8] (parity interleave)
                    ks = smallp.tile([128, UC], F32, tag="ks")
                    kev = ks[:, 0:1]
                    kev_s = bass_mod.AP(tensor=kev.tensor, offset=kev.offset,
                                        ap=[kev.ap[0], [2, 4], [1, 1]])
                    nc.vector.tensor_copy(out=kev_s, in_=s_ev[:, :, E:E + 1])
                    kod = ks[:, 1:2]
                    kod_s = bass_mod.AP(tensor=kod.tensor, offset=kod.offset,
                                        ap=[kod.ap[0], [2, 4], [1, 1]])
                    nc.vector.tensor_copy(out=kod_s, in_=s_od[:, :, E:E + 1])

                    # ksum exclusive prefix along buckets -> per-bucket scale R
                    ksc = smallp.tile([128, UC], F32, tag=f"ksc{p}")
                    nc.vector.tensor_tensor_scan(
                        out=ksc[:], data0=ks[:], data1=ks[:],
                        initial=c_ms[:, p, E:E + 1], op0=OP.add, op1=OP.bypass)
                    rs = smallp.tile([128, UC], F32, tag="rs")
                    nc.vector.tensor_copy(out=rs[:, 1:UC], in_=ksc[:, 0:UC - 1])
                    nc.vector.tensor_copy(out=rs[:, 0:1], in_=c_ms[:, p, E:E + 1])
                    R = smallp.tile([128, UC], BF16, tag="R")
                    nc.vector.tensor_scalar_add(rs[:], rs[:], EPS)
                    with nc.allow_low_precision(reason="bf16 ksum recip, 4e-3 rel"):
                        nc.vector.reciprocal(out=R[:], in_=rs[:])
                    if c == 0:
                        nc.vector.memset(R[:, 0:1], 0.0)  # bucket-0 blindspot

                    # chunk context total -> running carry master (the scan and
                    # rs reads above already took the pre-chunk state; c_bd
                    # still holds it for the attn matmuls of this chunk)
                    red_ev = smallp.tile([128, E + 1], F32, tag="red_ev")
                    nc.vector.tensor_reduce(
                        out=red_ev[:], in_=s_ev[:].rearrange("p a e1 -> p e1 a"),
                        axis=mybir.AxisListType.X, op=OP.add)
                    red_od = smallp.tile([128, E + 1], F32, tag="red_od")
                    nc.vector.tensor_reduce(
                        out=red_od[:], in_=s_od[:].rearrange("p a e1 -> p e1 a"),
                        axis=mybir.AxisListType.X, op=OP.add)
                    nc.vector.tensor_tensor(out=c_ms[:, p, 0:E], in0=c_ms[:, p, 0:E],
                                            in1=red_ev[:, 0:E], op=OP.add)
                    nc.vector.tensor_tensor(out=c_ms[:, p, 0:E], in0=c_ms[:, p, 0:E],
                                            in1=red_od[:, 0:E], op=OP.add)
                    nc.vector.tensor_copy(out=c_ms[:, p, E:E + 1], in_=ksc[:, UC - 1:UC])

                    # q'' = exp(q) * softmax_recip * ksum_recip
                    rap = R[:]
                    Rb = bass_mod.AP(tensor=rap.tensor, offset=rap.offset,
                                     ap=[rap.ap[0], rap.ap[1], [0, BUCKET]])
                    RR = tmpp.tile([128, CH], BF16, tag="RR")
                    nc.vector.tensor_tensor(
                        out=RR[:].rearrange("p (u t) -> p u t", t=BUCKET),
                        in0=repl[:].rearrange("p (u t) -> p u t", t=BUCKET),
                        in1=Rb, op=OP.mult)
                    q2 = tmpp.tile([128, CH], BF16, tag=f"q2{p}")
                    nc.vector.tensor_tensor(out=q2[:], in0=E_sb[:, p, :], in1=RR[:],
                                            op=OP.mult)
                    st[f"q2{p}"] = q2

            def emit_attn(c):
                # attn^T accumulation per pair; the q2/scan chains were
                # resolved a whole q-projection ago.  Block-diagonal lhsT
                # covers both heads in one matmul.
                st = state[c]
                atn = actp.tile([128, PAIRS, CH], BF16, tag="atn")
                st["atn"] = atn
                for p in range(PAIRS):
                    q2 = st[f"q2{p}"]
                    at = psattn.tile([128, CH], F32, tag="attn")
                    nc.tensor.matmul(at[:], c_bd[:, p, :], q2[:],
                                     start=True, stop=False)
                    for j in range(UC - 1):
                        q0 = (j + 1) * BUCKET
                        nc.tensor.matmul(
                            at[:, q0:CH], s_bd[:, p, j, :], q2[:, q0:CH],
                            start=False, stop=(j == UC - 2))
                    nc.scalar.activation(out=atn[:, p, :], in_=at[:], func=AF.Copy)

                    # refresh the block-diagonal carry copy for the NEXT chunk
                    # (c_ms already advanced during emit_kv)
                    nc.vector.tensor_copy(out=c_bd[0:64, p, 0:64],
                                          in_=c_ms[0:64, p, 0:E])
                    nc.vector.tensor_copy(out=c_bd[64:128, p, 64:128],
                                          in_=c_ms[64:128, p, 0:E])

            def emit_out(c):
                st = state.pop(c)
                atn = st["atn"]
                osb = outp.tile([128, PAIRS, DIM], BF16, tag="osb")
                for tt in range(PAIRS):
                    for half in range(2):
                        op_ = psproj.tile([128, 512], F32, tag="proj")
                        for p in range(PAIRS):
                            nc.tensor.matmul(
                                op_[:], atn[:, p, tt * 128:(tt + 1) * 128],
                                wo_sb[:, p, half * 512:(half + 1) * 512],
                                start=(p == 0), stop=(p == PAIRS - 1))
                        nc.scalar.activation(
                            out=osb[:, tt, half * 512:(half + 1) * 512],
                            in_=op_[:], func=AF.Copy)
                nc.gpsimd.dma_start(
                    out=out[c * CH:(c + 1) * CH, :].rearrange(
                        "(tt p) d -> p tt d", p=128),
                    in_=osb[:])

            for c in range(n_chunks):
                emit_q(c)
                if c >= 1:
                    emit_attn(c - 1)
                emit_kv(c)
                if c >= 1:
                    emit_out(c - 1)
            emit_attn(n_chunks - 1)
            emit_out(n_chunks - 1)

    nc.finalize()
    return nc


def _orp():
    m = np.zeros((128, 128), dtype=ml_dtypes.bfloat16)
    for p in range(PAIRS):
        m[32 * p, 0:64] = 1
        m[32 * p + 1, 64:128] = 1
    return m


def build_in_maps(x, Wq, Wk, Wv, Wo):
    bf = ml_dtypes.bfloat16
    x = np.asarray(x)
    Wq, Wk, Wv, Wo = (np.asarray(w) for w in (Wq, Wk, Wv, Wo))
    in_maps = []
    for c in range(8):
        b, g = c // 2, c % 2
        sl = slice(g * F, (g + 1) * F)
        in_maps.append({
            "x": np.ascontiguousarray(x[b].T).astype(bf),
            "wq": Wq[:, sl].astype(bf),
            "wk": Wk[:, sl].astype(bf),
            "wv": Wv[:, sl].astype(bf),
            "wo": Wo[sl, :].astype(bf),
            "orp": _orp(),
        })
    return in_maps


def kernel(x, Wq, Wk, Wv, Wo, bo):
    from concourse.bass_utils import run_bass_kernel_spmd

    if "nc" not in _NC_CACHE:
        _NC_CACHE["nc"] = build_nc()
    nc = _NC_CACHE["nc"]

    in_maps = build_in_maps(x, Wq, Wk, Wv, Wo)
    res = run_bass_kernel_spmd(nc, in_maps, core_ids=list(range(8)))
    outs = [res.results[c]["out"].astype(np.float32) for c in range(8)]
    full = np.stack([outs[2 * b] + outs[2 * b + 1] for b in range(B)], axis=0)
    return (full + np.asarray(bo)[None, None, :].astype(np.float32)).astype(np.float32)


# revision 26
# speedup vs baseline: 1.3730x; 1.0146x over previous
"""Trainium2 Bass kernel for bucketed causal linear self-attention.

Model (B=4, T=4096, DIM=1024, H=16 heads, E=64, BUCKET=64):
  q,k,v = x@Wq, x@Wk, x@Wv ; q softmaxed over head-dim, k -> elu(k)+1
  per-bucket context C_u = cumsum_u(k_bu^T v_bu), normalized by cumsum of
  key-sums, shifted one bucket; attn_bu = q_bu @ C_{u-1}; out = attn@Wo + bo.

Sharding over 8 cores: core c -> batch c//2, head-group c%2 (8 heads = 512
feats). q/k/v column-sharded by head, Wo row-sharded; host sums the two
partial outputs per batch (all-reduce on host) and adds bo.

v5 structure per core:
  x arrives HOST-TRANSPOSED [DIM, T]; x^T tiles load as plain per-kt DMAs
  on the sync HWDGE ring.  attn^T accumulates on the tensor engine:
      C_carry^T @ q2  +  sum_j S_j^T @ q2[:, buckets > j]
  (shrinking-N matmuls; no per-bucket DVE walk).  psi = elu(k)+1 =
  min(exp(k),1) + relu(k): two ACT ops off the PSUM + one DVE stt, so the
  projection PSUM recycles fast.  Chunk emission is software-pipelined:
      q-proj(c) | attn-mms(c-1) | kv-proj+S+scan-chains(c) | out-proj(c-1)
  so every cross-engine chain resolves under dense unrelated PE work.
  Softmax reciprocal: one DVE reciprocal per chunk; per-token broadcast
  via a 2-row matmul.  q'' = exp(q)*softmax_recip*1/(ksum_prefix+eps),
  bucket-0 blindspot via a zeroed scale column on chunk 0.  Output bf16;
  host sums the two per-batch partials in f32 and adds bo.
"""

import sys
import numpy as np
import ml_dtypes

sys.path.insert(0, "/opt/trn_rl_repo")

B, T, DIM, H, BUCKET = 4, 4096, 1024, 16, 64
E = 64           # head dim
HC = 8           # heads per core
F = HC * E       # per-core feature width = 512
CH = 512         # tokens per chunk
UC = CH // BUCKET  # buckets per chunk = 8
PAIRS = HC // 2  # head pairs = 4
KT = DIM // 128  # contraction tiles = 8
EPS = 1e-6

_NC_CACHE = {}


def build_nc(n_chunks=T // CH):
    import concourse.bass as bass_mod
    import concourse.mybir as mybir
    from concourse import bacc
    from concourse.tile import TileContext

    BF16 = mybir.dt.bfloat16
    F32 = mybir.dt.float32
    AF = mybir.ActivationFunctionType
    OP = mybir.AluOpType

    Tt = n_chunks * CH

    nc = bacc.Bacc("TRN2", target_bir_lowering=False, debug=False, num_devices=8)
    x = nc.dram_tensor("x", [DIM, Tt], BF16, kind="ExternalInput").ap()
    wq = nc.dram_tensor("wq", [DIM, F], BF16, kind="ExternalInput").ap()
    wk = nc.dram_tensor("wk", [DIM, F], BF16, kind="ExternalInput").ap()
    wv = nc.dram_tensor("wv", [DIM, F], BF16, kind="ExternalInput").ap()
    wo = nc.dram_tensor("wo", [F, DIM], BF16, kind="ExternalInput").ap()
    orp = nc.dram_tensor("orp", [128, 128], BF16, kind="ExternalInput").ap()
    out = nc.dram_tensor("out", [Tt, DIM], BF16, kind="ExternalOutput").ap()

    with TileContext(nc) as tc:
        with tc.tile_pool(name="const", bufs=1) as constp, \
             tc.tile_pool(name="xt", bufs=2) as xtp, \
             tc.tile_pool(name="act", bufs=2) as actp, \
             tc.tile_pool(name="tmp", bufs=3) as tmpp, \
             tc.tile_pool(name="small", bufs=8) as smallp, \
             tc.tile_pool(name="outp", bufs=2) as outp, \
             tc.tile_pool(name="ps_proj", bufs=2, space="PSUM") as psproj, \
             tc.tile_pool(name="ps_s", bufs=1, space="PSUM") as pss, \
             tc.tile_pool(name="ps_attn", bufs=2, space="PSUM") as psattn, \
             tc.tile_pool(name="ps_misc", bufs=2, space="PSUM") as psmisc:

            # ---- resident constants ----
            wq_sb = constp.tile([128, KT, F], BF16, tag="wq")
            wk_sb = constp.tile([128, KT, F], BF16, tag="wk")
            wv_sb = constp.tile([128, KT, F], BF16, tag="wv")
            wo_sb = constp.tile([128, PAIRS, DIM], BF16, tag="wo")
            # wq on the scalar HWDGE ring so it lands in parallel with x^T
            # (sync ring) and the rest (gpsimd ring); per-kt pieces so the
            # first q matmul starts after 128KB, not 1MB
            for kt in range(KT):
                nc.scalar.dma_start(out=wq_sb[:, kt, :],
                                    in_=wq[kt * 128:(kt + 1) * 128, :])
            nc.gpsimd.dma_start(out=wv_sb[:], in_=wv.rearrange("(kt p) f -> p kt f", p=128))
            nc.gpsimd.dma_start(out=wo_sb[:], in_=wo.rearrange("(ft p) n -> p ft n", p=128))

            ones_sum = constp.tile([128, 2], BF16, tag="ones_sum")
            nc.vector.memset(ones_sum[:], 0.0)
            nc.vector.memset(ones_sum[0:64, 0:1], 1.0)
            nc.vector.memset(ones_sum[64:128, 1:2], 1.0)
            # orp[32p+0, 0:64]=1, orp[32p+1, 64:128]=1 (host-built)
            orp_sb = constp.tile([128, 128], BF16, tag="orp")
            nc.gpsimd.dma_start(out=orp_sb[:], in_=orp[:])

            # running context (+ ksum col 64) per pair, f32 master + a bf16
            # BLOCK-DIAGONAL copy (head A in rows 0:64 x cols 0:64, head B in
            # rows 64:128 x cols 64:128) so one matmul covers both heads
            c_ms = constp.tile([128, PAIRS, E + 1], F32, tag="c_ms")
            nc.vector.memset(c_ms[:], 0.0)
            c_bd = constp.tile([128, PAIRS, 128], BF16, tag="c_bd")
            nc.vector.memset(c_bd[:], 0.0)
            # per-bucket context in the same block-diagonal form (j = 0..6;
            # bucket 7 never feeds attn); zeros off-diagonal, set once
            s_bd = constp.tile([128, PAIRS, UC - 1, 128], BF16, tag="s_bd")
            nc.vector.memset(s_bd[:], 0.0)

            state = {}

            def emit_q(c):
                xT = xtp.tile([128, KT, CH], BF16, tag="xT")
                for kt in range(KT):
                    nc.sync.dma_start(
                        out=xT[:, kt, :],
                        in_=x[kt * 128:(kt + 1) * 128, c * CH:(c + 1) * CH])
                if c == 0:
                    # wk per-kt on the sync ring right behind chunk-0's x^T
                    # pieces, so the first kv block isn't gated by the
                    # serialized gpsimd ring
                    for kt in range(KT):
                        nc.sync.dma_start(out=wk_sb[:, kt, :],
                                          in_=wk[kt * 128:(kt + 1) * 128, :])

                # q^T, exp(q), per-token softmax sums (rows 32p..32p+2 of sm)
                E_sb = actp.tile([128, PAIRS, CH], BF16, tag="E")
                sm = psmisc.tile([128, CH], F32, tag="misc")
                for p in range(PAIRS):
                    qt = psproj.tile([128, CH], F32, tag="proj")
                    for kt in range(KT):
                        nc.tensor.matmul(qt[:], wq_sb[:, kt, p * 128:(p + 1) * 128],
                                         xT[:, kt, :], start=(kt == 0), stop=(kt == KT - 1))
                    nc.scalar.activation(out=E_sb[:, p, :], in_=qt[:], func=AF.Exp)
                    nc.tensor.matmul(sm[32 * p:32 * p + 2, :], ones_sum[:], E_sb[:, p, :],
                                     start=True, stop=True, tile_position=(0, 32 * p))
                state[c] = {"xT": xT, "E": E_sb, "sm": sm}

            def emit_kv(c):
                st = state[c]
                xT, E_sb, sm = st["xT"], st["E"], st.pop("sm")
                # softmax reciprocal emitted here so its 3.4us DVE occupancy
                # queues after the previous chunk's attn-phase casts
                recip_sb = actp.tile([128, CH], BF16, tag="recip")
                with nc.allow_low_precision(reason="bf16 softmax recip, 4e-3 rel"):
                    nc.vector.reciprocal(out=recip_sb[0:98, :], in_=sm[0:98, :])
                psik = actp.tile([128, PAIRS, CH], BF16, tag="psik")
                v_sb = actp.tile([128, PAIRS, HC * (E + 1) // PAIRS * PAIRS], BF16, tag="v")
                # v_sb free layout per tok-tile: 8 heads x 65 (64 v + ones col)
                for tt in range(PAIRS):  # 4 token tiles of 128
                    kp = psproj.tile([128, F], F32, tag="proj")
                    for kt in range(KT):
                        nc.tensor.matmul(kp[:], xT[:, kt, tt * 128:(tt + 1) * 128],
                                         wk_sb[:, kt, :], start=(kt == 0), stop=(kt == KT - 1))
                    # psi = elu(k)+1 = min(exp(k),1) + relu(k); both ACT ops
                    # read the PSUM so it recycles fast
                    ek = tmpp.tile([128, F], BF16, tag="ek")
                    nc.scalar.activation(out=ek[:], in_=kp[:], func=AF.Exp)
                    rk = tmpp.tile([128, F], BF16, tag="rk")
                    nc.scalar.activation(out=rk[:], in_=kp[:], func=AF.Relu)
                    nc.vector.scalar_tensor_tensor(
                        out=psik[:, tt, :], in0=ek[:], scalar=1.0, in1=rk[:],
                        op0=OP.min, op1=OP.add)

                    vp = psproj.tile([128, F], F32, tag="proj")
                    for kt in range(KT):
                        nc.tensor.matmul(vp[:], xT[:, kt, tt * 128:(tt + 1) * 128],
                                         wv_sb[:, kt, :], start=(kt == 0), stop=(kt == KT - 1))
                    v3 = v_sb[:, tt, :].rearrange("p (h e1) -> p h e1", e1=E + 1)
                    nc.scalar.activation(
                        out=v3[:, :, 0:E],
                        in_=vp[:].rearrange("p (h e) -> p h e", e=E), func=AF.Copy)
                    nc.vector.memset(v3[:, :, E:E + 1], 1.0)

                # per-bucket context matmuls + scan chains, all pairs
                for p in range(PAIRS):
                    # S_j = psi_bu^T @ [v_bu | 1]
                    s_ev = pss.tile([128, UC // 2, E + 1], F32, tag="s_ev")
                    s_od = pss.tile([128, UC // 2, E + 1], F32, tag="s_od")
                    for j in range(UC):
                        sdst = s_ev if j % 2 == 0 else s_od
                        tt, r0 = j // 2, (j % 2) * 64
                        for hh in range(2):
                            h = 2 * p + hh
                            nc.tensor.matmul(
                                sdst[hh * 64:(hh + 1) * 64, j // 2, :],
                                psik[r0:r0 + 64, tt, h * E:(h + 1) * E],
                                v_sb[r0:r0 + 64, tt, :].rearrange(
                                    "p (g e1) -> p g e1", e1=E + 1)[:, h, :],
                                start=True, stop=True,
                                tile_position=(r0, hh * 64))
                    # q'' broadcast matmul can go right behind the S block
                    repl = psmisc.tile([128, CH], F32, tag="misc")
                    nc.tensor.matmul(repl[:], orp_sb[32 * p:32 * p + 2, :],
                                     recip_sb[32 * p:32 * p + 2, :],
                                     start=True, stop=True, tile_position=(32 * p, 0))

                    # S_j -> bf16 block-diagonal [128, j, 128]: head A rows
                    # 0:64 x cols 0:64, head B rows 64:128 x cols 64:128.
                    # j parity interleave via stepped-stride APs.
                    for hh in range(2):
                        r0, col0 = hh * 64, hh * 64
                        dev = s_bd[r0:r0 + 64, p, 0:UC - 1, col0:col0 + 64]
                        dev_s = bass_mod.AP(tensor=dev.tensor, offset=dev.offset,
                                            ap=[dev.ap[0], [2 * 128, 4], dev.ap[2]])
                        nc.vector.tensor_copy(out=dev_s, in_=s_ev[r0:r0 + 64, :, 0:E])
                        dod = s_bd[r0:r0 + 64, p, 1:UC - 1, col0:col0 + 64]
                        dod_s = bass_mod.AP(tensor=dod.tensor, offset=dod.offset,
                                            ap=[dod.ap[0], [2 * 128, 3], dod.ap[2]])
                        nc.vector.tensor_copy(out=dod_s, in_=s_od[r0:r0 + 64, 0:3, 0:E])

                    # per-bucket key sums -> [128, 8] (parity interleave)
                    ks = smallp.tile([128, UC], F32, tag="ks")
                    kev = ks[:, 0:1]
                    kev_s = bass_mod.AP(tensor=kev.tensor, offset=kev.offset,
                                        ap=[kev.ap[0], [2, 4], [1, 1]])
                    nc.vector.tensor_copy(out=kev_s, in_=s_ev[:, :, E:E + 1])
                    kod = ks[:, 1:2]
                    kod_s = bass_mod.AP(tensor=kod.tensor, offset=kod.offset,
                                        ap=[kod.ap[0], [2, 4], [1, 1]])
                    nc.vector.tensor_copy(out=kod_s, in_=s_od[:, :, E:E + 1])

                    # ksum exclusive prefix along buckets -> per-bucket scale R
                    ksc = smallp.tile([128, UC], F32, tag=f"ksc{p}")
                    nc.vector.tensor_tensor_scan(
                        out=ksc[:], data0=ks[:], data1=ks[:],
                        initial=c_ms[:, p, E:E + 1], op0=OP.add, op1=OP.bypass)
                    rs = smallp.tile([128, UC], F32, tag="rs")
                    nc.vector.tensor_copy(out=rs[:, 1:UC], in_=ksc[:, 0:UC - 1])
                    nc.vector.tensor_copy(out=rs[:, 0:1], in_=c_ms[:, p, E:E + 1])
                    R = smallp.tile([128, UC], BF16, tag="R")
                    nc.vector.tensor_scalar_add(rs[:], rs[:], EPS)
                    with nc.allow_low_precision(reason="bf16 ksum recip, 4e-3 rel"):
                        nc.vector.reciprocal(out=R[:], in_=rs[:])
                    if c == 0:
                        nc.vector.memset(R[:, 0:1], 0.0)  # bucket-0 blindspot

                    # chunk context total -> running carry master (the scan and
                    # rs reads above already took the pre-chunk state; c_bd
                    # still holds it for the attn matmuls of this chunk)
                    red_ev = smallp.tile([128, E + 1], F32, tag="red_ev")
                    nc.vector.tensor_reduce(
                        out=red_ev[:], in_=s_ev[:].rearrange("p a e1 -> p e1 a"),
                        axis=mybir.AxisListType.X, op=OP.add)
                    red_od = smallp.tile([128, E + 1], F32, tag="red_od")
                    nc.vector.tensor_reduce(
                        out=red_od[:], in_=s_od[:].rearrange("p a e1 -> p e1 a"),
                        axis=mybir.AxisListType.X, op=OP.add)
                    nc.vector.tensor_tensor(out=c_ms[:, p, 0:E], in0=c_ms[:, p, 0:E],
                                            in1=red_ev[:, 0:E], op=OP.add)
                    nc.vector.tensor_tensor(out=c_ms[:, p, 0:E], in0=c_ms[:, p, 0:E],
                                            in1=red_od[:, 0:E], op=OP.add)
                    nc.vector.tensor_copy(out=c_ms[:, p, E:E + 1], in_=ksc[:, UC - 1:UC])

                    # q'' = exp(q) * softmax_recip * ksum_recip
                    rap = R[:]
                    Rb = bass_mod.AP(tensor=rap.tensor, offset=rap.offset,
                                     ap=[rap.ap[0], rap.ap[1], [0, BUCKET]])
                    RR = tmpp.tile([128, CH], BF16, tag="RR")
                    nc.vector.tensor_tensor(
                        out=RR[:].rearrange("p (u t) -> p u t", t=BUCKET),
                        in0=repl[:].rearrange("p (u t) -> p u t", t=BUCKET),
                        in1=Rb, op=OP.mult)
                    q2 = tmpp.tile([128, CH], BF16, tag=f"q2{p}")
                    nc.vector.tensor_tensor(out=q2[:], in0=E_sb[:, p, :], in1=RR[:],
                                            op=OP.mult)
                    st[f"q2{p}"] = q2

            def emit_attn(c):
                # attn^T accumulation per pair; the q2/scan chains were
                # resolved a whole q-projection ago.  Block-diagonal lhsT
                # covers both heads in one matmul.
                st = state[c]
                atn = actp.tile([128, PAIRS, CH], BF16, tag="atn")
                st["atn"] = atn
                for p in range(PAIRS):
                    q2 = st[f"q2{p}"]
                    at = psattn.tile([128, CH], F32, tag="attn")
                    nc.tensor.matmul(at[:], c_bd[:, p, :], q2[:],
                                     start=True, stop=False)
                    for j in range(UC - 1):
                        q0 = (j + 1) * BUCKET
                        nc.tensor.matmul(
                            at[:, q0:CH], s_bd[:, p, j, :], q2[:, q0:CH],
                            start=False, stop=(j == UC - 2))
                    nc.scalar.activation(out=atn[:, p, :], in_=at[:], func=AF.Copy)

                    # refresh the block-diagonal carry copy for the NEXT chunk
                    # (c_ms already advanced during emit_kv)
                    nc.vector.tensor_copy(out=c_bd[0:64, p, 0:64],
                                          in_=c_ms[0:64, p, 0:E])
                    nc.vector.tensor_copy(out=c_bd[64:128, p, 64:128],
                                          in_=c_ms[64:128, p, 0:E])

            def emit_out(c):
                st = state.pop(c)
                atn = st["atn"]
                osb = outp.tile([128, PAIRS, DIM], BF16, tag="osb")
                for tt in range(PAIRS):
                    for half in range(2):
                        op_ = psproj.tile([128, 512], F32, tag="proj")
                        for p in range(PAIRS):
                            nc.tensor.matmul(
                                op_[:], atn[:, p, tt * 128:(tt + 1) * 128],
                                wo_sb[:, p, half * 512:(half + 1) * 512],
                                start=(p == 0), stop=(p == PAIRS - 1))
                        nc.scalar.activation(
                            out=osb[:, tt, half * 512:(half + 1) * 512],
                            in_=op_[:], func=AF.Copy)
                nc.gpsimd.dma_start(
                    out=out[c * CH:(c + 1) * CH, :].rearrange(
                        "(tt p) d -> p tt d", p=128),
                    in_=osb[:])

            for c in range(n_chunks):
                emit_q(c)
                if c >= 1:
                    emit_attn(c - 1)
                emit_kv(c)
                if c >= 1:
                    emit_out(c - 1)
            emit_attn(n_chunks - 1)
            emit_out(n_chunks - 1)

    nc.finalize()
    return nc


def _orp():
    m = np.zeros((128, 128), dtype=ml_dtypes.bfloat16)
    for p in range(PAIRS):
        m[32 * p, 0:64] = 1
        m[32 * p + 1, 64:128] = 1
    return m


def build_in_maps(x, Wq, Wk, Wv, Wo):
    bf = ml_dtypes.bfloat16
    x = np.asarray(x)
    Wq, Wk, Wv, Wo = (np.asarray(w) for w in (Wq, Wk, Wv, Wo))
    in_maps = []
    for c in range(8):
        b, g = c // 2, c % 2
        sl = slice(g * F, (g + 1) * F)
        in_maps.append({
            "x": np.ascontiguousarray(x[b].T).astype(bf),
            "wq": Wq[:, sl].astype(bf),
            "wk": Wk[:, sl].astype(bf),
            "wv": Wv[:, sl].astype(bf),
            "wo": Wo[sl, :].astype(bf),
            "orp": _orp(),
        })
    return in_maps


def kernel(x, Wq, Wk, Wv, Wo, bo):
    from concourse.bass_utils import run_bass_kernel_spmd

    if "nc" not in _NC_CACHE:
        _NC_CACHE["nc"] = build_nc()
    nc = _NC_CACHE["nc"]

    in_maps = build_in_maps(x, Wq, Wk, Wv, Wo)
    res = run_bass_kernel_spmd(nc, in_maps, core_ids=list(range(8)))
    outs = [res.results[c]["out"].astype(np.float32) for c in range(8)]
    full = np.stack([outs[2 * b] + outs[2 * b + 1] for b in range(B)], axis=0)
    return (full + np.asarray(bo)[None, None, :].astype(np.float32)).astype(np.float32)


# revision 28
# speedup vs baseline: 1.3778x; 1.0035x over previous
"""Trainium2 Bass kernel for bucketed causal linear self-attention.

Model (B=4, T=4096, DIM=1024, H=16 heads, E=64, BUCKET=64):
  q,k,v = x@Wq, x@Wk, x@Wv ; q softmaxed over head-dim, k -> elu(k)+1
  per-bucket context C_u = cumsum_u(k_bu^T v_bu), normalized by cumsum of
  key-sums, shifted one bucket; attn_bu = q_bu @ C_{u-1}; out = attn@Wo + bo.

Sharding over 8 cores: core c -> batch c//2, head-group c%2 (8 heads = 512
feats). q/k/v column-sharded by head, Wo row-sharded; host sums the two
partial outputs per batch (all-reduce on host) and adds bo.

v5 structure per core:
  x arrives HOST-TRANSPOSED [DIM, T]; x^T tiles load as plain per-kt DMAs
  on the sync HWDGE ring.  attn^T accumulates on the tensor engine:
      C_carry^T @ q2  +  sum_j S_j^T @ q2[:, buckets > j]
  (shrinking-N matmuls; no per-bucket DVE walk).  psi = elu(k)+1 =
  min(exp(k),1) + relu(k): two ACT ops off the PSUM + one DVE stt, so the
  projection PSUM recycles fast.  Chunk emission is software-pipelined:
      q-proj(c) | attn-mms(c-1) | kv-proj+S+scan-chains(c) | out-proj(c-1)
  so every cross-engine chain resolves under dense unrelated PE work.
  Softmax reciprocal: one DVE reciprocal per chunk; per-token broadcast
  via a 2-row matmul.  q'' = exp(q)*softmax_recip*1/(ksum_prefix+eps),
  bucket-0 blindspot via a zeroed scale column on chunk 0.  Output bf16;
  host sums the two per-batch partials in f32 and adds bo.
"""

import sys
import numpy as np
import ml_dtypes

sys.path.insert(0, "/opt/trn_rl_repo")

B, T, DIM, H, BUCKET = 4, 4096, 1024, 16, 64
E = 64           # head dim
HC = 8           # heads per core
F = HC * E       # per-core feature width = 512
CH = 512         # tokens per chunk
UC = CH // BUCKET  # buckets per chunk = 8
PAIRS = HC // 2  # head pairs = 4
KT = DIM // 128  # contraction tiles = 8
EPS = 1e-6

_NC_CACHE = {}


def build_nc(n_chunks=T // CH):
    import concourse.bass as bass_mod
    import concourse.mybir as mybir
    from concourse import bacc
    from concourse.tile import TileContext

    BF16 = mybir.dt.bfloat16
    F32 = mybir.dt.float32
    AF = mybir.ActivationFunctionType
    OP = mybir.AluOpType

    Tt = n_chunks * CH

    nc = bacc.Bacc("TRN2", target_bir_lowering=False, debug=False, num_devices=8)
    x = nc.dram_tensor("x", [DIM, Tt], BF16, kind="ExternalInput").ap()
    wq = nc.dram_tensor("wq", [DIM, F], BF16, kind="ExternalInput").ap()
    wk = nc.dram_tensor("wk", [DIM, F], BF16, kind="ExternalInput").ap()
    wv = nc.dram_tensor("wv", [DIM, F], BF16, kind="ExternalInput").ap()
    wo = nc.dram_tensor("wo", [F, DIM], BF16, kind="ExternalInput").ap()
    orp = nc.dram_tensor("orp", [128, 128], BF16, kind="ExternalInput").ap()
    out = nc.dram_tensor("out", [Tt, DIM], BF16, kind="ExternalOutput").ap()

    with TileContext(nc) as tc:
        with tc.tile_pool(name="const", bufs=1) as constp, \
             tc.tile_pool(name="xt", bufs=2) as xtp, \
             tc.tile_pool(name="act", bufs=2) as actp, \
             tc.tile_pool(name="tmp", bufs=3) as tmpp, \
             tc.tile_pool(name="small", bufs=8) as smallp, \
             tc.tile_pool(name="outp", bufs=2) as outp, \
             tc.tile_pool(name="ps_proj", bufs=2, space="PSUM") as psproj, \
             tc.tile_pool(name="ps_s", bufs=1, space="PSUM") as pss, \
             tc.tile_pool(name="ps_attn", bufs=2, space="PSUM") as psattn, \
             tc.tile_pool(name="ps_misc", bufs=2, space="PSUM") as psmisc:

            # ---- resident constants ----
            wq_sb = constp.tile([128, KT, F], BF16, tag="wq")
            wk_sb = constp.tile([128, KT, F], BF16, tag="wk")
            wv_sb = constp.tile([128, KT, F], BF16, tag="wv")
            wo_sb = constp.tile([128, PAIRS, DIM], BF16, tag="wo")
            # wq on the scalar HWDGE ring so it lands in parallel with x^T
            # (sync ring) and the rest (gpsimd ring); per-kt pieces so the
            # first q matmul starts after 128KB, not 1MB
            for h4 in range(2):
                k0 = h4 * (KT // 2)
                nc.scalar.dma_start(
                    out=wq_sb[:, k0:k0 + KT // 2, :],
                    in_=wq[k0 * 128:(k0 + KT // 2) * 128, :].rearrange(
                        "(kt p) f -> p kt f", p=128))
            nc.gpsimd.dma_start(out=wv_sb[:], in_=wv.rearrange("(kt p) f -> p kt f", p=128))
            nc.gpsimd.dma_start(out=wo_sb[:], in_=wo.rearrange("(ft p) n -> p ft n", p=128))

            ones_sum = constp.tile([128, 2], BF16, tag="ones_sum")
            nc.vector.memset(ones_sum[:], 0.0)
            nc.vector.memset(ones_sum[0:64, 0:1], 1.0)
            nc.vector.memset(ones_sum[64:128, 1:2], 1.0)
            # orp[32p+0, 0:64]=1, orp[32p+1, 64:128]=1 (host-built)
            orp_sb = constp.tile([128, 128], BF16, tag="orp")
            nc.gpsimd.dma_start(out=orp_sb[:], in_=orp[:])

            # running context (+ ksum col 64) per pair, f32 master + a bf16
            # BLOCK-DIAGONAL copy (head A in rows 0:64 x cols 0:64, head B in
            # rows 64:128 x cols 64:128) so one matmul covers both heads
            c_ms = constp.tile([128, PAIRS, E + 1], F32, tag="c_ms")
            nc.vector.memset(c_ms[:], 0.0)
            c_bd = constp.tile([128, PAIRS, 128], BF16, tag="c_bd")
            nc.vector.memset(c_bd[:], 0.0)
            # per-bucket context in the same block-diagonal form (j = 0..6;
            # bucket 7 never feeds attn); zeros off-diagonal, set once
            s_bd = constp.tile([128, PAIRS, UC - 1, 128], BF16, tag="s_bd")
            nc.vector.memset(s_bd[:], 0.0)

            state = {}

            def emit_q(c):
                xT = xtp.tile([128, KT, CH], BF16, tag="xT")
                for kt in range(KT):
                    nc.sync.dma_start(
                        out=xT[:, kt, :],
                        in_=x[kt * 128:(kt + 1) * 128, c * CH:(c + 1) * CH])
                if c == 0:
                    # wk per-kt on the sync ring right behind chunk-0's x^T
                    # pieces, so the first kv block isn't gated by the
                    # serialized gpsimd ring
                    for kt in range(KT):
                        nc.sync.dma_start(out=wk_sb[:, kt, :],
                                          in_=wk[kt * 128:(kt + 1) * 128, :])

                # q^T, exp(q), per-token softmax sums (rows 32p..32p+2 of sm)
                E_sb = actp.tile([128, PAIRS, CH], BF16, tag="E")
                sm = psmisc.tile([128, CH], F32, tag="misc")
                for p in range(PAIRS):
                    qt = psproj.tile([128, CH], F32, tag="proj")
                    for kt in range(KT):
                        nc.tensor.matmul(qt[:], wq_sb[:, kt, p * 128:(p + 1) * 128],
                                         xT[:, kt, :], start=(kt == 0), stop=(kt == KT - 1))
                    nc.scalar.activation(out=E_sb[:, p, :], in_=qt[:], func=AF.Exp)
                    nc.tensor.matmul(sm[32 * p:32 * p + 2, :], ones_sum[:], E_sb[:, p, :],
                                     start=True, stop=True, tile_position=(0, 32 * p))
                state[c] = {"xT": xT, "E": E_sb, "sm": sm}

            def emit_kv(c):
                st = state[c]
                xT, E_sb, sm = st["xT"], st["E"], st.pop("sm")
                # softmax reciprocal emitted here so its 3.4us DVE occupancy
                # queues after the previous chunk's attn-phase casts
                recip_sb = actp.tile([128, CH], BF16, tag="recip")
                with nc.allow_low_precision(reason="bf16 softmax recip, 4e-3 rel"):
                    nc.vector.reciprocal(out=recip_sb[0:98, :], in_=sm[0:98, :])
                psik = actp.tile([128, PAIRS, CH], BF16, tag="psik")
                v_sb = actp.tile([128, PAIRS, HC * (E + 1) // PAIRS * PAIRS], BF16, tag="v")
                # v_sb free layout per tok-tile: 8 heads x 65 (64 v + ones col)
                for tt in range(PAIRS):  # 4 token tiles of 128
                    kp = psproj.tile([128, F], F32, tag="proj")
                    for kt in range(KT):
                        nc.tensor.matmul(kp[:], xT[:, kt, tt * 128:(tt + 1) * 128],
                                         wk_sb[:, kt, :], start=(kt == 0), stop=(kt == KT - 1))
                    # psi = elu(k)+1 = min(exp(k),1) + relu(k); both ACT ops
                    # read the PSUM so it recycles fast
                    ek = tmpp.tile([128, F], BF16, tag="ek")
                    nc.scalar.activation(out=ek[:], in_=kp[:], func=AF.Exp)
                    rk = tmpp.tile([128, F], BF16, tag="rk")
                    nc.scalar.activation(out=rk[:], in_=kp[:], func=AF.Relu)
                    nc.vector.scalar_tensor_tensor(
                        out=psik[:, tt, :], in0=ek[:], scalar=1.0, in1=rk[:],
                        op0=OP.min, op1=OP.add)

                    vp = psproj.tile([128, F], F32, tag="proj")
                    for kt in range(KT):
                        nc.tensor.matmul(vp[:], xT[:, kt, tt * 128:(tt + 1) * 128],
                                         wv_sb[:, kt, :], start=(kt == 0), stop=(kt == KT - 1))
                    v3 = v_sb[:, tt, :].rearrange("p (h e1) -> p h e1", e1=E + 1)
                    nc.scalar.activation(
                        out=v3[:, :, 0:E],
                        in_=vp[:].rearrange("p (h e) -> p h e", e=E), func=AF.Copy)
                    nc.vector.memset(v3[:, :, E:E + 1], 1.0)

                # per-bucket context matmuls + scan chains, all pairs
                for p in range(PAIRS):
                    # S_j = psi_bu^T @ [v_bu | 1]
                    s_ev = pss.tile([128, UC // 2, E + 1], F32, tag="s_ev")
                    s_od = pss.tile([128, UC // 2, E + 1], F32, tag="s_od")
                    for j in range(UC):
                        sdst = s_ev if j % 2 == 0 else s_od
                        tt, r0 = j // 2, (j % 2) * 64
                        for hh in range(2):
                            h = 2 * p + hh
                            nc.tensor.matmul(
                                sdst[hh * 64:(hh + 1) * 64, j // 2, :],
                                psik[r0:r0 + 64, tt, h * E:(h + 1) * E],
                                v_sb[r0:r0 + 64, tt, :].rearrange(
                                    "p (g e1) -> p g e1", e1=E + 1)[:, h, :],
                                start=True, stop=True,
                                tile_position=(r0, hh * 64))
                    # q'' broadcast matmul can go right behind the S block
                    repl = psmisc.tile([128, CH], F32, tag="misc")
                    nc.tensor.matmul(repl[:], orp_sb[32 * p:32 * p + 2, :],
                                     recip_sb[32 * p:32 * p + 2, :],
                                     start=True, stop=True, tile_position=(32 * p, 0))

                    # S_j -> bf16 block-diagonal [128, j, 128]: head A rows
                    # 0:64 x cols 0:64, head B rows 64:128 x cols 64:128.
                    # j parity interleave via stepped-stride APs.
                    for hh in range(2):
                        r0, col0 = hh * 64, hh * 64
                        # head A casts on DVE, head B on the scalar engine to
                        # keep the DVE FIFO ahead of the S matmuls
                        eng = nc.vector.tensor_copy if hh == 0 else None
                        dev = s_bd[r0:r0 + 64, p, 0:UC - 1, col0:col0 + 64]
                        dev_s = bass_mod.AP(tensor=dev.tensor, offset=dev.offset,
                                            ap=[dev.ap[0], [2 * 128, 4], dev.ap[2]])
                        dod = s_bd[r0:r0 + 64, p, 1:UC - 1, col0:col0 + 64]
                        dod_s = bass_mod.AP(tensor=dod.tensor, offset=dod.offset,
                                            ap=[dod.ap[0], [2 * 128, 3], dod.ap[2]])
                        if hh == 0:
                            nc.vector.tensor_copy(out=dev_s, in_=s_ev[r0:r0 + 64, :, 0:E])
                            nc.vector.tensor_copy(out=dod_s, in_=s_od[r0:r0 + 64, 0:3, 0:E])
                        else:
                            nc.scalar.activation(out=dev_s, in_=s_ev[r0:r0 + 64, :, 0:E],
                                                 func=AF.Copy)
                            nc.scalar.activation(out=dod_s, in_=s_od[r0:r0 + 64, 0:3, 0:E],
                                                 func=AF.Copy)

                    # per-bucket key sums -> [128, 8] (parity interleave)
                    ks = smallp.tile([128, UC], F32, tag="ks")
                    kev = ks[:, 0:1]
                    kev_s = bass_mod.AP(tensor=kev.tensor, offset=kev.offset,
                                        ap=[kev.ap[0], [2, 4], [1, 1]])
                    nc.vector.tensor_copy(out=kev_s, in_=s_ev[:, :, E:E + 1])
                    kod = ks[:, 1:2]
                    kod_s = bass_mod.AP(tensor=kod.tensor, offset=kod.offset,
                                        ap=[kod.ap[0], [2, 4], [1, 1]])
                    nc.vector.tensor_copy(out=kod_s, in_=s_od[:, :, E:E + 1])

                    # ksum exclusive prefix along buckets -> per-bucket scale R
                    ksc = smallp.tile([128, UC], F32, tag=f"ksc{p}")
                    nc.vector.tensor_tensor_scan(
                        out=ksc[:], data0=ks[:], data1=ks[:],
                        initial=c_ms[:, p, E:E + 1], op0=OP.add, op1=OP.bypass)
                    rs = smallp.tile([128, UC], F32, tag="rs")
                    nc.vector.tensor_copy(out=rs[:, 1:UC], in_=ksc[:, 0:UC - 1])
                    nc.vector.tensor_copy(out=rs[:, 0:1], in_=c_ms[:, p, E:E + 1])
                    R = smallp.tile([128, UC], BF16, tag="R")
                    nc.vector.tensor_scalar_add(rs[:], rs[:], EPS)
                    with nc.allow_low_precision(reason="bf16 ksum recip, 4e-3 rel"):
                        nc.vector.reciprocal(out=R[:], in_=rs[:])
                    if c == 0:
                        nc.vector.memset(R[:, 0:1], 0.0)  # bucket-0 blindspot

                    # chunk context total -> running carry master (the scan and
                    # rs reads above already took the pre-chunk state; c_bd
                    # still holds it for the attn matmuls of this chunk)
                    red_ev = smallp.tile([128, E + 1], F32, tag="red_ev")
                    nc.vector.tensor_reduce(
                        out=red_ev[:], in_=s_ev[:].rearrange("p a e1 -> p e1 a"),
                        axis=mybir.AxisListType.X, op=OP.add)
                    red_od = smallp.tile([128, E + 1], F32, tag="red_od")
                    nc.vector.tensor_reduce(
                        out=red_od[:], in_=s_od[:].rearrange("p a e1 -> p e1 a"),
                        axis=mybir.AxisListType.X, op=OP.add)
                    nc.vector.tensor_tensor(out=c_ms[:, p, 0:E], in0=c_ms[:, p, 0:E],
                                            in1=red_ev[:, 0:E], op=OP.add)
                    nc.vector.tensor_tensor(out=c_ms[:, p, 0:E], in0=c_ms[:, p, 0:E],
                                            in1=red_od[:, 0:E], op=OP.add)
                    nc.vector.tensor_copy(out=c_ms[:, p, E:E + 1], in_=ksc[:, UC - 1:UC])

                    # q'' = exp(q) * softmax_recip * ksum_recip
                    rap = R[:]
                    Rb = bass_mod.AP(tensor=rap.tensor, offset=rap.offset,
                                     ap=[rap.ap[0], rap.ap[1], [0, BUCKET]])
                    RR = tmpp.tile([128, CH], BF16, tag="RR")
                    nc.vector.tensor_tensor(
                        out=RR[:].rearrange("p (u t) -> p u t", t=BUCKET),
                        in0=repl[:].rearrange("p (u t) -> p u t", t=BUCKET),
                        in1=Rb, op=OP.mult)
                    q2 = tmpp.tile([128, CH], BF16, tag=f"q2{p}")
                    nc.vector.tensor_tensor(out=q2[:], in0=E_sb[:, p, :], in1=RR[:],
                                            op=OP.mult)
                    st[f"q2{p}"] = q2

            def emit_attn(c):
                # attn^T accumulation per pair; the q2/scan chains were
                # resolved a whole q-projection ago.  Block-diagonal lhsT
                # covers both heads in one matmul.
                st = state[c]
                atn = actp.tile([128, PAIRS, CH], BF16, tag="atn")
                st["atn"] = atn
                for p in range(PAIRS):
                    q2 = st[f"q2{p}"]
                    at = psattn.tile([128, CH], F32, tag="attn")
                    nc.tensor.matmul(at[:], c_bd[:, p, :], q2[:],
                                     start=True, stop=False)
                    for j in range(UC - 1):
                        q0 = (j + 1) * BUCKET
                        nc.tensor.matmul(
                            at[:, q0:CH], s_bd[:, p, j, :], q2[:, q0:CH],
                            start=False, stop=(j == UC - 2))
                    nc.scalar.activation(out=atn[:, p, :], in_=at[:], func=AF.Copy)

                    # refresh the block-diagonal carry copy for the NEXT chunk
                    # (c_ms already advanced during emit_kv)
                    nc.vector.tensor_copy(out=c_bd[0:64, p, 0:64],
                                          in_=c_ms[0:64, p, 0:E])
                    nc.vector.tensor_copy(out=c_bd[64:128, p, 64:128],
                                          in_=c_ms[64:128, p, 0:E])

            def emit_out(c):
                st = state.pop(c)
                atn = st["atn"]
                osb = outp.tile([128, PAIRS, DIM], BF16, tag="osb")
                for tt in range(PAIRS):
                    for half in range(2):
                        op_ = psproj.tile([128, 512], F32, tag="proj")
                        for p in range(PAIRS):
                            nc.tensor.matmul(
                                op_[:], atn[:, p, tt * 128:(tt + 1) * 128],
                                wo_sb[:, p, half * 512:(half + 1) * 512],
                                start=(p == 0), stop=(p == PAIRS - 1))
                        nc.scalar.activation(
                            out=osb[:, tt, half * 512:(half + 1) * 512],
                            in_=op_[:], func=AF.Copy)
                nc.gpsimd.dma_start(
                    out=out[c * CH:(c + 1) * CH, :].rearrange(
                        "(tt p) d -> p tt d", p=128),
                    in_=osb[:])

            for c in range(n_chunks):
                emit_q(c)
                if c >= 1:
                    emit_attn(c - 1)
                emit_kv(c)
                if c >= 1:
                    emit_out(c - 1)
            emit_attn(n_chunks - 1)
            emit_out(n_chunks - 1)

    nc.finalize()
    return nc


def _orp():
    m = np.zeros((128, 128), dtype=ml_dtypes.bfloat16)
    for p in range(PAIRS):
        m[32 * p, 0:64] = 1
        m[32 * p + 1, 64:128] = 1
    return m


def build_in_maps(x, Wq, Wk, Wv, Wo):
    bf = ml_dtypes.bfloat16
    x = np.asarray(x)
    Wq, Wk, Wv, Wo = (np.asarray(w) for w in (Wq, Wk, Wv, Wo))
    in_maps = []
    for c in range(8):
        b, g = c // 2, c % 2
        sl = slice(g * F, (g + 1) * F)
        in_maps.append({
            "x": np.ascontiguousarray(x[b].T).astype(bf),
            "wq": Wq[:, sl].astype(bf),
            "wk": Wk[:, sl].astype(bf),
            "wv": Wv[:, sl].astype(bf),
            "wo": Wo[sl, :].astype(bf),
            "orp": _orp(),
        })
    return in_maps


def kernel(x, Wq, Wk, Wv, Wo, bo):
    from concourse.bass_utils import run_bass_kernel_spmd

    if "nc" not in _NC_CACHE:
        _NC_CACHE["nc"] = build_nc()
    nc = _NC_CACHE["nc"]

    in_maps = build_in_maps(x, Wq, Wk, Wv, Wo)
    res = run_bass_kernel_spmd(nc, in_maps, core_ids=list(range(8)))
    outs = [res.results[c]["out"].astype(np.float32) for c in range(8)]
    full = np.stack([outs[2 * b] + outs[2 * b + 1] for b in range(B)], axis=0)
    return (full + np.asarray(bo)[None, None, :].astype(np.float32)).astype(np.float32)


# revision 30
# speedup vs baseline: 1.3912x; 1.0098x over previous
"""Trainium2 Bass kernel for bucketed causal linear self-attention.

Model (B=4, T=4096, DIM=1024, H=16 heads, E=64, BUCKET=64):
  q,k,v = x@Wq, x@Wk, x@Wv ; q softmaxed over head-dim, k -> elu(k)+1
  per-bucket context C_u = cumsum_u(k_bu^T v_bu), normalized by cumsum of
  key-sums, shifted one bucket; attn_bu = q_bu @ C_{u-1}; out = attn@Wo + bo.

Sharding over 8 cores: core c -> batch c//2, head-group c%2 (8 heads = 512
feats). q/k/v column-sharded by head, Wo row-sharded; host sums the two
partial outputs per batch (all-reduce on host) and adds bo.

v5 structure per core:
  x arrives HOST-TRANSPOSED [DIM, T]; x^T tiles load as plain per-kt DMAs
  on the sync HWDGE ring.  attn^T accumulates on the tensor engine:
      C_carry^T @ q2  +  sum_j S_j^T @ q2[:, buckets > j]
  (shrinking-N matmuls; no per-bucket DVE walk).  psi = elu(k)+1 =
  min(exp(k),1) + relu(k): two ACT ops off the PSUM + one DVE stt, so the
  projection PSUM recycles fast.  Chunk emission is software-pipelined:
      q-proj(c) | attn-mms(c-1) | kv-proj+S+scan-chains(c) | out-proj(c-1)
  so every cross-engine chain resolves under dense unrelated PE work.
  Softmax reciprocal: one DVE reciprocal per chunk; per-token broadcast
  via a 2-row matmul.  q'' = exp(q)*softmax_recip*1/(ksum_prefix+eps),
  bucket-0 blindspot via a zeroed scale column on chunk 0.  Output bf16;
  host sums the two per-batch partials in f32 and adds bo.
"""

import sys
import numpy as np
import ml_dtypes

sys.path.insert(0, "/opt/trn_rl_repo")

B, T, DIM, H, BUCKET = 4, 4096, 1024, 16, 64
E = 64           # head dim
HC = 8           # heads per core
F = HC * E       # per-core feature width = 512
CH = 512         # tokens per chunk
UC = CH // BUCKET  # buckets per chunk = 8
PAIRS = HC // 2  # head pairs = 4
KT = DIM // 128  # contraction tiles = 8
EPS = 1e-6

_NC_CACHE = {}


def build_nc(n_chunks=T // CH):
    import concourse.bass as bass_mod
    import concourse.mybir as mybir
    from concourse import bacc
    from concourse.tile import TileContext

    BF16 = mybir.dt.bfloat16
    F32 = mybir.dt.float32
    AF = mybir.ActivationFunctionType
    OP = mybir.AluOpType

    Tt = n_chunks * CH

    nc = bacc.Bacc("TRN2", target_bir_lowering=False, debug=False, num_devices=8)
    x = nc.dram_tensor("x", [DIM, Tt], BF16, kind="ExternalInput").ap()
    wq = nc.dram_tensor("wq", [DIM, F], BF16, kind="ExternalInput").ap()
    wk = nc.dram_tensor("wk", [DIM, F], BF16, kind="ExternalInput").ap()
    wv = nc.dram_tensor("wv", [DIM, F], BF16, kind="ExternalInput").ap()
    wo = nc.dram_tensor("wo", [F, DIM], BF16, kind="ExternalInput").ap()
    orp = nc.dram_tensor("orp", [128, 128], BF16, kind="ExternalInput").ap()
    out = nc.dram_tensor("out", [Tt, DIM], BF16, kind="ExternalOutput").ap()

    with TileContext(nc) as tc:
        with tc.tile_pool(name="const", bufs=1) as constp, \
             tc.tile_pool(name="xt", bufs=2) as xtp, \
             tc.tile_pool(name="act", bufs=2) as actp, \
             tc.tile_pool(name="tmp", bufs=3) as tmpp, \
             tc.tile_pool(name="small", bufs=8) as smallp, \
             tc.tile_pool(name="outp", bufs=2) as outp, \
             tc.tile_pool(name="ps_proj", bufs=2, space="PSUM") as psproj, \
             tc.tile_pool(name="ps_s", bufs=1, space="PSUM") as pss, \
             tc.tile_pool(name="ps_attn", bufs=2, space="PSUM") as psattn, \
             tc.tile_pool(name="ps_misc", bufs=2, space="PSUM") as psmisc:

            # ---- resident constants ----
            wq_sb = constp.tile([128, KT, F], BF16, tag="wq")
            wk_sb = constp.tile([128, KT, F], BF16, tag="wk")
            wv_sb = constp.tile([128, KT, F], BF16, tag="wv")
            wo_sb = constp.tile([128, PAIRS, DIM], BF16, tag="wo")
            # wq on the scalar HWDGE ring so it lands in parallel with x^T
            # (sync ring) and the rest (gpsimd ring); per-kt pieces so the
            # first q matmul starts after 128KB, not 1MB
            for h4 in range(2):
                k0 = h4 * (KT // 2)
                nc.scalar.dma_start(
                    out=wq_sb[:, k0:k0 + KT // 2, :],
                    in_=wq[k0 * 128:(k0 + KT // 2) * 128, :].rearrange(
                        "(kt p) f -> p kt f", p=128))
            nc.gpsimd.dma_start(out=wv_sb[:], in_=wv.rearrange("(kt p) f -> p kt f", p=128))
            nc.gpsimd.dma_start(out=wo_sb[:], in_=wo.rearrange("(ft p) n -> p ft n", p=128))

            ones_sum = constp.tile([128, 2], BF16, tag="ones_sum")
            nc.vector.memset(ones_sum[:], 0.0)
            nc.vector.memset(ones_sum[0:64, 0:1], 1.0)
            nc.vector.memset(ones_sum[64:128, 1:2], 1.0)
            # orp[32p+0, 0:64]=1, orp[32p+1, 64:128]=1 (host-built)
            orp_sb = constp.tile([128, 128], BF16, tag="orp")
            nc.gpsimd.dma_start(out=orp_sb[:], in_=orp[:])

            # running context (+ ksum col 64) per pair, f32 master + a bf16
            # BLOCK-DIAGONAL copy (head A in rows 0:64 x cols 0:64, head B in
            # rows 64:128 x cols 64:128) so one matmul covers both heads
            c_ms = constp.tile([128, PAIRS, E + 1], F32, tag="c_ms")
            nc.vector.memset(c_ms[:], 0.0)
            c_bd = constp.tile([128, PAIRS, 128], BF16, tag="c_bd")
            nc.vector.memset(c_bd[:], 0.0)
            # per-bucket context in the same block-diagonal form (j = 0..6;
            # bucket 7 never feeds attn); zeros off-diagonal, set once
            s_bd = constp.tile([128, PAIRS, UC - 1, 128], BF16, tag="s_bd")
            nc.vector.memset(s_bd[:], 0.0)

            state = {}

            def emit_q(c):
                xT = xtp.tile([128, KT, CH], BF16, tag="xT")
                for kt in range(KT):
                    nc.sync.dma_start(
                        out=xT[:, kt, :],
                        in_=x[kt * 128:(kt + 1) * 128, c * CH:(c + 1) * CH])
                if c == 0:
                    # wk per-kt on the sync ring right behind chunk-0's x^T
                    # pieces, so the first kv block isn't gated by the
                    # serialized gpsimd ring
                    for kt in range(KT):
                        nc.sync.dma_start(out=wk_sb[:, kt, :],
                                          in_=wk[kt * 128:(kt + 1) * 128, :])

                # q^T, exp(q), per-token softmax sums (rows 32p..32p+2 of sm)
                E_sb = actp.tile([128, PAIRS, CH], BF16, tag="E")
                sm = psmisc.tile([128, CH], F32, tag="misc")
                for p in range(PAIRS):
                    qt = psproj.tile([128, CH], F32, tag="proj")
                    for kt in range(KT):
                        nc.tensor.matmul(qt[:], wq_sb[:, kt, p * 128:(p + 1) * 128],
                                         xT[:, kt, :], start=(kt == 0), stop=(kt == KT - 1))
                    nc.scalar.activation(out=E_sb[:, p, :], in_=qt[:], func=AF.Exp)
                    nc.tensor.matmul(sm[32 * p:32 * p + 2, :], ones_sum[:], E_sb[:, p, :],
                                     start=True, stop=True, tile_position=(0, 32 * p))
                state[c] = {"xT": xT, "E": E_sb, "sm": sm}

            def emit_kv(c):
                st = state[c]
                xT, E_sb, sm = st["xT"], st["E"], st.pop("sm")
                # softmax reciprocal emitted here so its 3.4us DVE occupancy
                # queues after the previous chunk's attn-phase casts
                recip_sb = actp.tile([128, CH], BF16, tag="recip")
                with nc.allow_low_precision(reason="bf16 softmax recip, 4e-3 rel"):
                    nc.vector.reciprocal(out=recip_sb[0:98, :], in_=sm[0:98, :])
                psik = actp.tile([128, PAIRS, CH], BF16, tag="psik")
                v_sb = actp.tile([128, PAIRS, HC * (E + 1) // PAIRS * PAIRS], BF16, tag="v")
                # v_sb free layout per tok-tile: 8 heads x 65 (64 v + ones col)
                for tt in range(PAIRS):  # 4 token tiles of 128
                    kp = psproj.tile([128, F], F32, tag="proj")
                    for kt in range(KT):
                        nc.tensor.matmul(kp[:], xT[:, kt, tt * 128:(tt + 1) * 128],
                                         wk_sb[:, kt, :], start=(kt == 0), stop=(kt == KT - 1))
                    # psi = elu(k)+1 = min(exp(k),1) + relu(k); both ACT ops
                    # read the PSUM so it recycles fast
                    ek = tmpp.tile([128, F], BF16, tag="ek")
                    nc.scalar.activation(out=ek[:], in_=kp[:], func=AF.Exp)
                    rk = tmpp.tile([128, F], BF16, tag="rk")
                    nc.scalar.activation(out=rk[:], in_=kp[:], func=AF.Relu)
                    nc.vector.scalar_tensor_tensor(
                        out=psik[:, tt, :], in0=ek[:], scalar=1.0, in1=rk[:],
                        op0=OP.min, op1=OP.add)

                    vp = psproj.tile([128, F], F32, tag="proj")
                    for kt in range(KT):
                        nc.tensor.matmul(vp[:], xT[:, kt, tt * 128:(tt + 1) * 128],
                                         wv_sb[:, kt, :], start=(kt == 0), stop=(kt == KT - 1))
                    v3 = v_sb[:, tt, :].rearrange("p (h e1) -> p h e1", e1=E + 1)
                    nc.scalar.activation(
                        out=v3[:, :, 0:E],
                        in_=vp[:].rearrange("p (h e) -> p h e", e=E), func=AF.Copy)
                    nc.vector.memset(v3[:, :, E:E + 1], 1.0)

                # per-bucket context matmuls + scan chains, all pairs
                for p in range(PAIRS):
                    # S_j = psi_bu^T @ [v_bu | 1]
                    s_ev = pss.tile([128, UC // 2, E + 1], F32, tag="s_ev")
                    s_od = pss.tile([128, UC // 2, E + 1], F32, tag="s_od")
                    for j in range(UC):
                        sdst = s_ev if j % 2 == 0 else s_od
                        tt, r0 = j // 2, (j % 2) * 64
                        for hh in range(2):
                            h = 2 * p + hh
                            nc.tensor.matmul(
                                sdst[hh * 64:(hh + 1) * 64, j // 2, :],
                                psik[r0:r0 + 64, tt, h * E:(h + 1) * E],
                                v_sb[r0:r0 + 64, tt, :].rearrange(
                                    "p (g e1) -> p g e1", e1=E + 1)[:, h, :],
                                start=True, stop=True,
                                tile_position=(r0, hh * 64))
                    # q'' broadcast matmul can go right behind the S block
                    repl = psmisc.tile([128, CH], F32, tag="misc")
                    nc.tensor.matmul(repl[:], orp_sb[32 * p:32 * p + 2, :],
                                     recip_sb[32 * p:32 * p + 2, :],
                                     start=True, stop=True, tile_position=(32 * p, 0))

                    # S_j -> bf16 block-diagonal [128, j, 128]: head A rows
                    # 0:64 x cols 0:64, head B rows 64:128 x cols 64:128.
                    # j parity interleave via stepped-stride APs.
                    for hh in range(2):
                        r0, col0 = hh * 64, hh * 64
                        # head A casts on DVE, head B on the scalar engine to
                        # keep the DVE FIFO ahead of the S matmuls
                        eng = nc.vector.tensor_copy if hh == 0 else None
                        dev = s_bd[r0:r0 + 64, p, 0:UC - 1, col0:col0 + 64]
                        dev_s = bass_mod.AP(tensor=dev.tensor, offset=dev.offset,
                                            ap=[dev.ap[0], [2 * 128, 4], dev.ap[2]])
                        dod = s_bd[r0:r0 + 64, p, 1:UC - 1, col0:col0 + 64]
                        dod_s = bass_mod.AP(tensor=dod.tensor, offset=dod.offset,
                                            ap=[dod.ap[0], [2 * 128, 3], dod.ap[2]])
                        if hh == 0:
                            nc.vector.tensor_copy(out=dev_s, in_=s_ev[r0:r0 + 64, :, 0:E])
                            nc.vector.tensor_copy(out=dod_s, in_=s_od[r0:r0 + 64, 0:3, 0:E])
                        else:
                            nc.scalar.activation(out=dev_s, in_=s_ev[r0:r0 + 64, :, 0:E],
                                                 func=AF.Copy)
                            nc.scalar.activation(out=dod_s, in_=s_od[r0:r0 + 64, 0:3, 0:E],
                                                 func=AF.Copy)

                    # per-bucket key sums -> [128, 8] (parity interleave)
                    ks = smallp.tile([128, UC], F32, tag="ks")
                    kev = ks[:, 0:1]
                    kev_s = bass_mod.AP(tensor=kev.tensor, offset=kev.offset,
                                        ap=[kev.ap[0], [2, 4], [1, 1]])
                    nc.vector.tensor_copy(out=kev_s, in_=s_ev[:, :, E:E + 1])
                    kod = ks[:, 1:2]
                    kod_s = bass_mod.AP(tensor=kod.tensor, offset=kod.offset,
                                        ap=[kod.ap[0], [2, 4], [1, 1]])
                    nc.vector.tensor_copy(out=kod_s, in_=s_od[:, :, E:E + 1])

                    # ksum exclusive prefix along buckets -> per-bucket scale R
                    ksc = smallp.tile([128, UC], F32, tag=f"ksc{p}")
                    nc.vector.tensor_tensor_scan(
                        out=ksc[:], data0=ks[:], data1=ks[:],
                        initial=c_ms[:, p, E:E + 1], op0=OP.add, op1=OP.bypass)
                    rs = smallp.tile([128, UC], F32, tag="rs")
                    nc.vector.tensor_copy(out=rs[:, 1:UC], in_=ksc[:, 0:UC - 1])
                    nc.vector.tensor_copy(out=rs[:, 0:1], in_=c_ms[:, p, E:E + 1])
                    R = smallp.tile([128, UC], BF16, tag="R")
                    nc.vector.tensor_scalar_add(rs[:], rs[:], EPS)
                    with nc.allow_low_precision(reason="bf16 ksum recip, 4e-3 rel"):
                        nc.vector.reciprocal(out=R[:], in_=rs[:])
                    if c == 0:
                        nc.vector.memset(R[:, 0:1], 0.0)  # bucket-0 blindspot

                    # chunk context total -> running carry master (the scan and
                    # rs reads above already took the pre-chunk state; c_bd
                    # still holds it for the attn matmuls of this chunk)
                    red_ev = smallp.tile([128, E + 1], F32, tag="red_ev")
                    nc.vector.tensor_reduce(
                        out=red_ev[:], in_=s_ev[:].rearrange("p a e1 -> p e1 a"),
                        axis=mybir.AxisListType.X, op=OP.add)
                    red_od = smallp.tile([128, E + 1], F32, tag="red_od")
                    nc.vector.tensor_reduce(
                        out=red_od[:], in_=s_od[:].rearrange("p a e1 -> p e1 a"),
                        axis=mybir.AxisListType.X, op=OP.add)
                    nc.vector.tensor_tensor(out=c_ms[:, p, 0:E], in0=c_ms[:, p, 0:E],
                                            in1=red_ev[:, 0:E], op=OP.add)
                    nc.vector.tensor_tensor(out=c_ms[:, p, 0:E], in0=c_ms[:, p, 0:E],
                                            in1=red_od[:, 0:E], op=OP.add)
                    nc.vector.tensor_copy(out=c_ms[:, p, E:E + 1], in_=ksc[:, UC - 1:UC])

                    # q'' = exp(q) * softmax_recip * ksum_recip
                    rap = R[:]
                    Rb = bass_mod.AP(tensor=rap.tensor, offset=rap.offset,
                                     ap=[rap.ap[0], rap.ap[1], [0, BUCKET]])
                    RR = tmpp.tile([128, CH], BF16, tag="RR")
                    nc.vector.tensor_tensor(
                        out=RR[:].rearrange("p (u t) -> p u t", t=BUCKET),
                        in0=repl[:].rearrange("p (u t) -> p u t", t=BUCKET),
                        in1=Rb, op=OP.mult)
                    q2 = tmpp.tile([128, CH], BF16, tag=f"q2{p}")
                    nc.vector.tensor_tensor(out=q2[:], in0=E_sb[:, p, :], in1=RR[:],
                                            op=OP.mult)
                    st[f"q2{p}"] = q2

            def emit_attn(c):
                # attn^T accumulation per pair; the q2/scan chains were
                # resolved a whole q-projection ago.  Block-diagonal lhsT
                # covers both heads in one matmul.
                st = state[c]
                atn = actp.tile([128, PAIRS, CH], BF16, tag="atn")
                st["atn"] = atn
                for p in range(PAIRS):
                    q2 = st[f"q2{p}"]
                    at = psattn.tile([128, CH], F32, tag="attn")
                    nc.tensor.matmul(at[:], c_bd[:, p, :], q2[:],
                                     start=True, stop=False)
                    for j in range(UC - 1):
                        q0 = (j + 1) * BUCKET
                        nc.tensor.matmul(
                            at[:, q0:CH], s_bd[:, p, j, :], q2[:, q0:CH],
                            start=False, stop=(j == UC - 2))
                    nc.scalar.activation(out=atn[:, p, :], in_=at[:], func=AF.Copy)

                    # refresh the block-diagonal carry copy for the NEXT chunk
                    # (c_ms already advanced during emit_kv)
                    nc.vector.tensor_copy(out=c_bd[0:64, p, 0:64],
                                          in_=c_ms[0:64, p, 0:E])
                    nc.vector.tensor_copy(out=c_bd[64:128, p, 64:128],
                                          in_=c_ms[64:128, p, 0:E])

            def emit_out(c):
                st = state.pop(c)
                atn = st["atn"]
                osb = outp.tile([128, PAIRS, DIM], BF16, tag="osb")
                for tt in range(PAIRS):
                    for half in range(2):
                        op_ = psproj.tile([128, 512], F32, tag="proj")
                        for p in range(PAIRS):
                            nc.tensor.matmul(
                                op_[:], atn[:, p, tt * 128:(tt + 1) * 128],
                                wo_sb[:, p, half * 512:(half + 1) * 512],
                                start=(p == 0), stop=(p == PAIRS - 1))
                        nc.scalar.activation(
                            out=osb[:, tt, half * 512:(half + 1) * 512],
                            in_=op_[:], func=AF.Copy)
                nc.gpsimd.dma_start(
                    out=out[c * CH:(c + 1) * CH, :].rearrange(
                        "(tt p) d -> p tt d", p=128),
                    in_=osb[:])

            for c in range(n_chunks):
                emit_q(c)
                if c >= 1:
                    emit_attn(c - 1)
                emit_kv(c)
                if c >= 1:
                    emit_out(c - 1)
            emit_attn(n_chunks - 1)
            emit_out(n_chunks - 1)

    nc.finalize()
    return nc


def _orp():
    m = np.zeros((128, 128), dtype=ml_dtypes.bfloat16)
    for p in range(PAIRS):
        m[32 * p, 0:64] = 1
        m[32 * p + 1, 64:128] = 1
    return m


def build_in_maps(x, Wq, Wk, Wv, Wo):
    bf = ml_dtypes.bfloat16
    x = np.asarray(x)
    Wq, Wk, Wv, Wo = (np.asarray(w) for w in (Wq, Wk, Wv, Wo))
    in_maps = []
    for c in range(8):
        b, g = c // 2, c % 2
        sl = slice(g * F, (g + 1) * F)
        in_maps.append({
            "x": np.ascontiguousarray(x[b].T).astype(bf),
            "wq": Wq[:, sl].astype(bf),
            "wk": Wk[:, sl].astype(bf),
            "wv": Wv[:, sl].astype(bf),
            "wo": Wo[sl, :].astype(bf),
            "orp": _orp(),
        })
    return in_maps


def kernel(x, Wq, Wk, Wv, Wo, bo):
    from concourse.bass_utils import run_bass_kernel_spmd

    if "nc" not in _NC_CACHE:
        _NC_CACHE["nc"] = build_nc()
    nc = _NC_CACHE["nc"]

    in_maps = build_in_maps(x, Wq, Wk, Wv, Wo)
    res = run_bass_kernel_spmd(nc, in_maps, core_ids=list(range(8)))
    outs = [res.results[c]["out"].astype(np.float32) for c in range(8)]
    full = np.stack([outs[2 * b] + outs[2 * b + 1] for b in range(B)], axis=0)
    return (full + np.asarray(bo)[None, None, :].astype(np.float32)).astype(np.float32)
